# revision 32
# baseline (speedup 1.0000x reference)
"""Trainium2 Bass kernel: LocalCausalTransformerBlock (window-3 causal attention).

Sharding: 8-way sequence-parallel. B=2 x N=2048 = 4096 tokens -> 8 chunks of
512 tokens (4 chunks per batch row). Each core gets its 512 tokens plus a
2-token halo (the preceding tokens of the same sequence) so the window-3
causal attention needs no cross-core communication. Weights are replicated.

The four big matmuls (qkv/proj/fc1/fc2) run in fp8e4m3 with DoubleRow perf
mode (contract 2x128 channels per instruction at 0.5 cycles/row). Weights are
pre-scaled per output column to a power of two near absmax~2 so e4m3's
subnormal range is never hit; the descale rides the eviction's free
multiplicative scalar slot. qkv and proj additionally carry the quantization
residual ("lo") as extra fp8 k-chunks in the same accumulation group.
Attention internals (q/k/v, softmax, AV) are bf16; LayerNorm stats, softmax
normalizer and both residual streams are fp32.

Layout: activations live "transposed" (channels on partitions, tokens on the
free axis) so matmuls contract over partitions and the +-1/+-2 token shifts
of the local attention are free-axis offsets. LayerNorm runs token-major; PE
transposes bridge the layouts, batched 8-to-a-psum-bank with one wide strided
eviction. Softmax needs no max-subtraction (window-3 scores are small): exp
runs directly on the score PSUM; a per-core multiplicative edge mask zeroes
out-of-window columns after exp. Act-table funcs are ordered
sqrt->exp->sqrt->gelu (4 loads; identity is in every set). Weight matrices
stream in as column-block DMAs so matmuls start before the full matrix
lands; small constants ride in two packed DMAs. PSUM evictions are spread
across DVE and Act (gpsimd cannot touch PSUM); Pool takes SBUF-only work.
"""

import sys

for _p in ("/opt/trn_rl_repo",):
    if _p not in sys.path:
        sys.path.insert(0, _p)

import numpy as np
import ml_dtypes

P = 128
D = 1024
H = 16
HD = 64
H3 = 3 * D
HID = 4096
T = 512            # real tokens per core
TH = T + 2         # k/v token axis with 2-token halo (halo stored first)
NCORE = 8
EPS = 1e-5
BF = ml_dtypes.bfloat16
F8 = ml_dtypes.float8_e4m3

# which weights carry the fp8 quantization residual (2x k-chunks)
COMP = {"qkv": False, "proj": False, "fc1": False, "fc2": False}

# packed f32 const columns
_C = {}
_off = 0
for _name, _w in [("qkvb", 24), ("qkvs", 24), ("projb", 8), ("projs", 8),
                  ("fc1b", 32), ("fc1s", 32), ("fc2b", 8), ("fc2s", 8),
                  ("khs", 32), ("khb", 32)]:
    _C[_name] = _off
    _off += _w
CPAK_W = _off
# packed bf16 const columns: idb, hmask, emk
_B = {"idb": 0, "hmask": 128, "emk": 256}
BPAK_W = 260

_CACHE: dict = {}


def _build_program(bias_p=False, bias_f=False):
    """bias_p/bias_f: include ones-chunks in proj/fc2 matmuls to add a
    nonzero proj_b/fc2_b (the channel-major residual fusion has no other
    slot for them). Left off when the biases are zero."""
    import concourse.bass as bass
    import concourse.tile as tile
    from concourse import bacc, mybir, bass_isa
    from contextlib import ExitStack

    f32 = mybir.dt.float32
    bf16 = mybir.dt.bfloat16
    fp8 = mybir.dt.float8e4
    ALU = mybir.AluOpType
    ACT = mybir.ActivationFunctionType
    DR = mybir.MatmulPerfMode.DoubleRow

    KQ = 16 if COMP["qkv"] else 8
    KP = (16 if COMP["proj"] else 8) + (2 if bias_p else 0)
    K1 = 16 if COMP["fc1"] else 8
    K2 = (64 if COMP["fc2"] else 32) + (2 if bias_f else 0)
    NCH_A = 8 + (2 if bias_p else 0)   # attnT chunks (+ones pair)
    NCH_H = 32 + (2 if bias_f else 0)  # hT chunks (+ones pair)

    nc = bacc.Bacc()

    xh_d = nc.declare_dram_parameter("xh", [2, D], bf16, isOutput=False)
    xm_d = nc.declare_dram_parameter("xm", [T, D], bf16, isOutput=False)
    qkvw_ds = [nc.declare_dram_parameter(f"qkvw{b}", [P, KQ * 768], fp8,
                                         isOutput=False) for b in range(4)]
    projw_d = nc.declare_dram_parameter("projw", [P, KP * D], fp8, isOutput=False)
    fc1w_ds = [nc.declare_dram_parameter(f"fc1w{b}", [P, K1 * 2048], fp8,
                                         isOutput=False) for b in range(2)]
    fc2w_ds = [nc.declare_dram_parameter(f"fc2w{b}", [P, K2 * 512], fp8,
                                         isOutput=False) for b in range(2)]
    cpak_d = nc.declare_dram_parameter("cpak", [P, CPAK_W], f32, isOutput=False)
    bpak_d = nc.declare_dram_parameter("bpak", [P, BPAK_W], bf16, isOutput=False)
    out_d = nc.declare_dram_parameter("out", [T, D], bf16, isOutput=True)
    # DRAM scratch for the probs partition-broadcast round-trip
    pw_dram = nc.dram_tensor("pwd", (H, 3 * T), bf16, kind="Internal")

    with tile.TileContext(nc) as tc, ExitStack() as ctx:
        # PSUM budget (8 banks): mm x4, sc x2 (scores <-> fc2), tp x2
        const = ctx.enter_context(tc.tile_pool(name="const", bufs=1))
        acts = ctx.enter_context(tc.tile_pool(name="acts", bufs=1))
        ln_pool = ctx.enter_context(tc.tile_pool(name="ln", bufs=2))
        tp_ps = ctx.enter_context(tc.tile_pool(name="tp_ps", bufs=2, space="PSUM"))
        mm_ps = ctx.enter_context(tc.tile_pool(name="mm_ps", bufs=4, space="PSUM"))
        sc_ps = ctx.enter_context(tc.tile_pool(name="sc_ps", bufs=2, space="PSUM"))

        bpak = const.tile([P, BPAK_W], bf16, tag="bp", name="bpak")
        cpak = const.tile([P, CPAK_W], f32, tag="cp", name="cpak")

        def cp(name, j, w=1):
            o = _C[name] + j
            return cpak[:, o:o + w]

        idb = bpak[:, _B["idb"]:_B["idb"] + 128]
        hmask = bpak[:, _B["hmask"]:_B["hmask"] + 128]
        emk = bpak[0:H, _B["emk"]:_B["emk"] + 3]

        # activations alive into the MLP phases (channel-major residual)
        xT = acts.tile([P, 8, T], bf16, tag="xT", name="xT")
        x2T = acts.tile([P, 8, T], bf16, tag="x2T", name="x2T")
        x2lnT = acts.tile([P, 8, T], fp8, tag="x2lnT", name="x2lnT")
        # weights preloaded early so their DMAs overlap earlier phases
        projw = acts.tile([P, KP, D], fp8, tag="projw", name="projw")
        fc1w = [acts.tile([P, K1, 2048], fp8, tag=f"fc1w{b}", name=f"fc1w{b}")
                for b in range(2)]
        fc2w = [acts.tile([P, K2, 512], fp8, tag=f"fc2w{b}", name=f"fc2w{b}")
                for b in range(2)]

        def layernorm_tok(src_ap, s, dstT, dst_off, pool=None, tp_tag="tp",
                          tp_pool=None, split_apply=False, evict="act"):
            """Token-major LN over s tokens -> fp8 channel-major in
            dstT[:, ch, dst_off:dst_off+s]. Stats on DVE, rstd via Act sqrt +
            DVE reciprocal, apply on Pool (optionally split Pool/DVE),
            transpose batch on PE, one wide Act eviction."""
            pool = pool or ln_pool
            tp_pool = tp_pool or tp_ps
            stat = pool.tile([s, 12], bf16, tag=f"lnstat{s}", name=f"st{s}")
            nc.vector.bn_stats(stat[:, 0:6], src_ap[:, 0:512])
            nc.vector.bn_stats(stat[:, 6:12], src_ap[:, 512:1024])
            mv = pool.tile([s, 2], f32, tag=f"lnmv{s}", name=f"mv{s}")
            nc.vector.bn_aggr(mv[:], stat[:])
            vpe = pool.tile([s, 1], f32, tag=f"lnvpe{s}", name=f"vpe{s}")
            nc.vector.tensor_scalar_add(vpe[:], mv[:, 1:2], EPS)
            std = pool.tile([s, 1], f32, tag=f"lnstd{s}", name=f"sd{s}")
            nc.scalar.activation(std[:], vpe[:], ACT.Sqrt)
            rstd = pool.tile([s, 1], f32, tag=f"lnrstd{s}", name=f"rs{s}")
            nc.vector.reciprocal(rstd[:], std[:])
            nmr = pool.tile([s, 1], f32, tag=f"lnnmr{s}", name=f"nm{s}")
            nc.vector.scalar_tensor_tensor(
                nmr[:], mv[:, 0:1], -1.0, rstd[:], ALU.mult, ALU.mult
            )
            xln = pool.tile([s, D], bf16, tag=f"lnout{s}", name=f"xo{s}")
            if split_apply:
                nc.gpsimd.tensor_scalar(xln[:, 0:512], src_ap[:, 0:512],
                                        rstd[:, 0:1], nmr[:, 0:1],
                                        ALU.mult, ALU.add)
                nc.gpsimd.tensor_scalar(xln[:, 512:1024], src_ap[:, 512:1024],
                                        rstd[:, 0:1], nmr[:, 0:1],
                                        ALU.mult, ALU.add)
            else:
                nc.gpsimd.tensor_scalar(xln[:], src_ap[:], rstd[:, 0:1],
                                        nmr[:, 0:1], ALU.mult, ALU.add)
            tpw = tp_pool.tile([P, 8, s], bf16, tag=tp_tag, name=f"tpln{s}")
            for ch in range(8):
                nc.tensor.transpose(tpw[:, ch, :], xln[:, ch * P:(ch + 1) * P],
                                    idb[0:s, 0:s])
            if evict == "act":
                nc.scalar.activation(dstT[:, :, dst_off:dst_off + s], tpw[:],
                                     ACT.Identity)
            else:
                nc.vector.tensor_copy(dstT[:, :, dst_off:dst_off + s], tpw[:])

        with tc.tile_pool(name="p1", bufs=1) as p1:
            xt = p1.tile([P, 4 * D], bf16, tag="xt", name="xt")
            xh = p1.tile([2, D], bf16, tag="xh", name="xh")
            xlnT = p1.tile([P, 8, T], fp8, tag="xlnT", name="xlnT")
            xlnTh = p1.tile([P, 8, 2], fp8, tag="xlnTh", name="xlnTh")
            qT = p1.tile([P, 8 * T], bf16, tag="qT", name="qT")
            kvT = p1.tile([P, 16, TH], bf16, tag="kvT", name="kvT")

            # SP queue: halo + x first (LN1 critical), consts, then q-half
            # of the qkv weights, projw and fc2w. Act queue: k/v-half of
            # qkv weights (needed a bit later). Pool queue: fc1w, issued
            # after the LN1 applies so they don't block them.
            nc.sync.dma_start(xh[:], xh_d[:])
            for ti in range(4):
                nc.sync.dma_start(xt[:, ti * D:(ti + 1) * D],
                                  xm_d[ti * P:(ti + 1) * P, :])
            nc.sync.dma_start(bpak[:], bpak_d[:])
            nc.sync.dma_start(cpak[:], cpak_d[:])

            with tc.tile_pool(name="p3", bufs=1) as p3:
                attnT = p3.tile([P, NCH_A, T], fp8, tag="attnT",
                                name="attnT")
                if bias_p:
                    nc.vector.memset(attnT[:, 8, :], 1.0)
                    nc.vector.memzero(attnT[:, 9, :])
                with tc.tile_pool(name="p2", bufs=1) as p2:
                    et = p2.tile([H, 3, T], bf16, tag="et", name="et")
                    with tc.tile_pool(name="wq", bufs=1) as wq_pool:
                        qkvw = []
                        for b in range(4):
                            t = wq_pool.tile([P, KQ, 768], fp8,
                                             tag=f"qkvw{b}", name=f"qkvw{b}")
                            eng = nc.sync if b < 2 else nc.scalar
                            eng.dma_start(t[:], qkvw_ds[b][:])
                            qkvw.append(t)
                        nc.sync.dma_start(projw[:], projw_d[:])

                        for b in range(2):
                            nc.sync.dma_start(fc1w[b][:], fc1w_ds[b][:])
                        for b in range(2):
                            nc.sync.dma_start(fc2w[b][:], fc2w_ds[b][:])

                        # channel-major raw x for the residual stream
                        # (PE + Act are idle this early)
                        for ti in range(4):
                            tpx = tp_ps.tile([P, 8, P], bf16, tag="tp",
                                             name=f"tpx{ti}")
                            for ch in range(8):
                                nc.tensor.transpose(
                                    tpx[:, ch, :],
                                    xt[:, ti * D + ch * P:
                                       ti * D + (ch + 1) * P],
                                    idb[:, :])
                            nc.scalar.activation(
                                xT[:, :, ti * P:(ti + 1) * P], tpx[:],
                                ACT.Identity)

                        # ---- LN1 (halo first: xh lands first) ----
                        layernorm_tok(xh[:], 2, xlnTh, 0)
                        for ti in range(4):
                            layernorm_tok(xt[:, ti * D:(ti + 1) * D], P,
                                          xlnT, ti * P)

                        # ---- QKV ----
                        # halo k/v columns: one psum tile = 16 blocks x 2 cols
                        ph = tp_ps.tile([P, 8, 4], f32, tag="tp", name="ph")
                        for j in range(16):
                            col = D + j * P
                            wt = qkvw[col // 768]
                            wo = col % 768
                            for i in range(KQ // 2):
                                xc = (2 * i) % 8
                                nc.tensor.matmul(
                                    ph[:, j // 2, (j % 2) * 2:(j % 2) * 2 + 2],
                                    wt[:, 2 * i:2 * i + 2, wo:wo + P],
                                    xlnTh[:, xc:xc + 2, :],
                                    start=(i == 0), stop=(i == KQ // 2 - 1),
                                    perf_mode=DR,
                                )
                        pht = ln_pool.tile([P, 32], f32, tag="pht", name="pht")
                        nc.vector.tensor_mul(pht[:], ph[:, :, :],
                                             cp("khs", 0, 32))
                        for j in range(16):
                            nc.gpsimd.tensor_add(
                                kvT[:, j, 0:2], pht[:, 2 * j:2 * j + 2],
                                cp("khb", 2 * j, 2))

                        def qkv_tile(j):
                            wt = qkvw[j // 6]
                            wo = (j % 6) * P
                            ps = mm_ps.tile([P, T], f32, tag="mm",
                                            name=f"qkv{j}")
                            for i in range(KQ // 2):
                                xc = (2 * i) % 8
                                nc.tensor.matmul(
                                    ps[:], wt[:, 2 * i:2 * i + 2, wo:wo + P],
                                    xlnT[:, xc:xc + 2, :],
                                    start=(i == 0), stop=(i == KQ // 2 - 1),
                                    perf_mode=DR,
                                )
                            if j < 8:
                                dst = qT[:, j * T:(j + 1) * T]
                            else:
                                dst = kvT[:, j - 8, 2:TH]
                            if j % 2 == 0:
                                nc.vector.tensor_scalar(
                                    dst, ps[:], cp("qkvs", j), cp("qkvb", j),
                                    ALU.mult, ALU.add)
                            else:
                                nc.scalar.activation(dst, ps[:], ACT.Identity,
                                                     bias=cp("qkvb", j),
                                                     scale=cp("qkvs", j))

                        for j in range(16):      # q then k
                            qkv_tile(j)
                        # scores overlap the v-tile matmuls below
                        for w in range(3):
                            e = p2.tile([P, 4, T], bf16, tag="e", bufs=2,
                                        name=f"e{w}")
                            e2 = p2.tile([P, 4, T], bf16, tag="e", bufs=2,
                                         name=f"e2{w}")
                            nc.vector.tensor_mul(
                                e[:], qT[:, 0:4 * T],
                                kvT[:, 0:4, 2 - w:2 - w + T])
                            nc.vector.tensor_mul(
                                e2[:], qT[:, 4 * T:8 * T],
                                kvT[:, 4:8, 2 - w:2 - w + T])
                            sc = sc_ps.tile([H, T], f32, tag="sc",
                                            name=f"sc{w}")
                            for ch in range(8):
                                esrc = e if ch < 4 else e2
                                nc.tensor.matmul(
                                    sc[:], hmask[:, ch * H:(ch + 1) * H],
                                    esrc[:, ch % 4, :],
                                    start=(ch == 0), stop=(ch == 7),
                                )
                            nc.scalar.activation(et[:, w, :], sc[:], ACT.Exp)
                        # preload the sqrt act table for LN2 while Act
                        # has slack (identity is in every table)
                        scr = ln_pool.tile([P, 1], f32, tag="scr", name="scr")
                        nc.scalar.activation(scr[:], cp("qkvs", 0), ACT.Sqrt)
                        # ---- softmax (before the v evictions so pw is
                        # ready when the PE reaches the bc matmuls) ----
                        nc.gpsimd.tensor_mul(et[:, 1, 0:1], et[:, 1, 0:1],
                                             emk[:, 0:1])
                        nc.gpsimd.tensor_mul(et[:, 2, 0:2], et[:, 2, 0:2],
                                             emk[:, 1:3])
                        z0 = p2.tile([H, T], bf16, tag="z0", name="z0")
                        z1 = p2.tile([H, T], bf16, tag="z1", name="z1")
                        rz = p2.tile([H, T], bf16, tag="z0", name="rz")
                        nc.gpsimd.tensor_add(z0[:], et[:, 0, :], et[:, 1, :])
                        nc.gpsimd.tensor_add(z1[:], z0[:], et[:, 2, :])
                        with nc.allow_low_precision(reason="softmax bf16"):
                            nc.vector.reciprocal(rz[:], z1[:])
                        for w in range(3):
                            nc.vector.tensor_mul(et[:, w, :], et[:, w, :],
                                                 rz[:])
                        # probs partition-broadcast: SBUF -> DRAM, then ONE
                        # stride-0 DMA fans head rows out. Channels are
                        # host-permuted head-minor (head = partition // 8,
                        # identical in every chunk), so a single [128, 3T]
                        # tile serves all 8 chunks.
                        nc.scalar.dma_start(pw_dram[:], et[:, :, :])
                        bcs = p2.tile([P, 3, T], bf16, tag="bcs", name="bcs")
                        import concourse.ap as cap
                        src = cap.AP(pw_dram, 0,
                                     [[3 * T, H], [0, 8], [1, 3 * T]])
                        nc.scalar.dma_start(bcs[:], src)
                        for j in range(16, 24):  # v
                            qkv_tile(j)

                        for chp in range(4):  # chunk pairs, fully streamed
                            ch = 2 * chp
                            avs = []
                            for w in range(3):
                                av = p2.tile([P, 2, T], bf16, tag="av",
                                             bufs=4, name=f"av{chp}_{w}")
                                for c in range(2):
                                    nc.vector.tensor_mul(
                                        av[:, c, :], bcs[:, w, :],
                                        kvT[:, 8 + ch + c,
                                            2 - w:2 - w + T],
                                    )
                                avs.append(av)
                            av01 = p2.tile([P, 2, T], bf16, tag="av01",
                                           bufs=2, name=f"av01_{chp}")
                            eng = nc.vector if chp == 3 else nc.gpsimd
                            eng.tensor_add(av01[:], avs[0][:], avs[1][:])
                            eng.tensor_add(attnT[:, ch:ch + 2, :],
                                           av01[:], avs[2][:])

                # ---- proj + residual 1 + LN2 (all channel-major) ----
                with tc.tile_pool(name="p5", bufs=1) as p5:
                    # 8 concurrent psum groups streaming over attnT pairs
                    pjps = {}
                    for j in range(8):
                        pool, tag = [(sc_ps, "sc"), (mm_ps, "mm"),
                                     (tp_ps, "tp")][0 if j < 2 else
                                                    (1 if j < 6 else 2)]
                        pjps[j] = pool.tile([P, T], f32, tag=tag,
                                            name=f"pj{j}")
                    for i in range(KP // 2):
                        for j in range(8):
                            nc.tensor.matmul(
                                pjps[j][:], projw[:, 2 * i:2 * i + 2,
                                                  j * P:(j + 1) * P],
                                attnT[:, 2 * i:2 * i + 2, :],
                                start=(i == 0), stop=(i == KP // 2 - 1),
                                perf_mode=DR,
                            )
                    # fused evict + scale + residual: x2T = pj*s + xT
                    for j in range(8):
                        nc.vector.scalar_tensor_tensor(
                            x2T[:, j, :], pjps[j][:], cp("projs", j),
                            xT[:, j, :], ALU.mult, ALU.add)
                    # LN2 stats channel-major: pairwise folds + gpsimd
                    # partition all-reduce give per-token sums broadcast
                    # to every partition; row math runs on those tiles.
                    sq = p5.tile([P, 8, T], bf16, tag="sq", name="sq")
                    f1 = p5.tile([P, 4, T], bf16, tag="f1", name="f1")
                    g1t = p5.tile([P, 4, T], bf16, tag="g1t", name="g1t")
                    f2t = p5.tile([P, 2, T], bf16, tag="f2t", name="f2t")
                    g2t = p5.tile([P, 2, T], bf16, tag="g2t", name="g2t")
                    f3 = p5.tile([P, T], bf16, tag="f3", name="f3")
                    g3 = p5.tile([P, T], bf16, tag="g3", name="g3")
                    ars = p5.tile([P, T], f32, tag="ars", name="ars")
                    arq = p5.tile([P, T], f32, tag="arq", name="arq")
                    nc.vector.tensor_mul(sq[:], x2T[:], x2T[:])
                    nc.vector.tensor_add(f1[:], x2T[:, 0:4, :],
                                         x2T[:, 4:8, :])
                    nc.gpsimd.tensor_add(g1t[:], sq[:, 0:4, :],
                                         sq[:, 4:8, :])
                    nc.vector.tensor_add(f2t[:], f1[:, 0:2, :],
                                         f1[:, 2:4, :])
                    nc.gpsimd.tensor_add(g2t[:], g1t[:, 0:2, :],
                                         g1t[:, 2:4, :])
                    nc.vector.tensor_add(f3[:], f2t[:, 0, :], f2t[:, 1, :])
                    nc.gpsimd.tensor_add(g3[:], g2t[:, 0, :], g2t[:, 1, :])
                    nc.gpsimd.partition_all_reduce(ars[:], f3[:], P,
                                                   bass_isa.ReduceOp.add)
                    nc.gpsimd.partition_all_reduce(arq[:], g3[:], P,
                                                   bass_isa.ReduceOp.add)
                    mu = p5.tile([P, T], bf16, tag="mu", name="mu")
                    e2n = p5.tile([P, T], bf16, tag="e2n", name="e2n")
                    mu2 = p5.tile([P, T], bf16, tag="mu2", name="mu2")
                    var = p5.tile([P, T], bf16, tag="var", name="var")
                    stdt = p5.tile([P, T], bf16, tag="stdt", name="stdt")
                    rstd = p5.tile([P, T], bf16, tag="rstdb", name="rstdb")
                    mrs = p5.tile([P, T], bf16, tag="mrs", name="mrs")
                    nc.vector.tensor_scalar_mul(mu[:], ars[:], 1.0 / D)
                    nc.vector.tensor_scalar(e2n[:], arq[:], 1.0 / D, EPS,
                                            ALU.mult, ALU.add)
                    nc.gpsimd.tensor_mul(mu2[:], mu[:], mu[:])
                    nc.vector.tensor_sub(var[:], e2n[:], mu2[:])
                    nc.scalar.activation(stdt[:], var[:], ACT.Sqrt)
                    with nc.allow_low_precision(reason="ln2 bf16 rows"):
                        nc.vector.reciprocal(rstd[:], stdt[:])
                    nc.gpsimd.tensor_mul(mrs[:], mu[:], rstd[:])
                    t1 = p5.tile([P, 8, T], bf16, tag="t1", name="t1")
                    for ch in range(8):
                        ea = nc.vector if ch % 2 == 0 else nc.gpsimd
                        eb = nc.gpsimd if ch % 2 == 0 else nc.vector
                        ea.tensor_mul(t1[:, ch, :], x2T[:, ch, :], rstd[:])
                        eb.tensor_sub(x2lnT[:, ch, :], t1[:, ch, :], mrs[:])
                    scr2 = ln_pool.tile([P, 1], f32, tag="scr", name="scr2")
                    nc.scalar.activation(scr2[:], cp("qkvs", 0), ACT.Gelu)

        # ---- MLP fc1 + gelu, fc2 + residual 2 + store ----
        # fc1 tiles rotate on tp_ps; fc2 keeps 6 psum groups live on
        # mm_ps+sc_ps for the whole phase, its i-step lagging the fc1
        # round that produced those hT chunks by one round so the
        # in-order PE queue never stalls on a gelu eviction.
        with tc.tile_pool(name="w1", bufs=1) as w1_pool:
                outt = w1_pool.tile([P, 4 * D], bf16, tag="outt", name="outt")
                mT = w1_pool.tile([P, 8 * T], bf16, tag="mT", name="mT")
                hT = w1_pool.tile([P, NCH_H, T], fp8, tag="hT", name="hT")
                if bias_f:
                    nc.vector.memset(hT[:, 32, :], 1.0)
                    nc.vector.memzero(hT[:, 33, :])

                def f2_mm(ps, j, i):
                    wt = fc2w[j // 4]
                    wo = (j % 4) * P
                    nc.tensor.matmul(
                        ps[:], wt[:, 2 * i:2 * i + 2, wo:wo + P],
                        hT[:, 2 * i:2 * i + 2, :],
                        start=(i == 0), stop=(i == K2 // 2 - 1),
                        perf_mode=DR,
                    )

                # fused evict + scale + residual: out3 = f2*s + x2T
                def f2_evict(ps, j):
                    nc.vector.scalar_tensor_tensor(
                        mT[:, j * T:(j + 1) * T], ps[:], cp("fc2s", j),
                        x2T[:, j, :], ALU.mult, ALU.add)

                f2ps = {}
                for j in range(6):
                    pool = sc_ps if j < 2 else mm_ps
                    f2ps[j] = pool.tile([P, T], f32,
                                        tag="sc" if j < 2 else "mm",
                                        name=f"f2{j}")

                for r in range(16):
                    for jj in (2 * r, 2 * r + 1):
                        wt = fc1w[jj // 16]
                        wo = (jj % 16) * P
                        ps = tp_ps.tile([P, T], f32, tag="tp", name=f"f1{jj}")
                        for i in range(K1 // 2):
                            xc = (2 * i) % 8
                            nc.tensor.matmul(
                                ps[:], wt[:, 2 * i:2 * i + 2, wo:wo + P],
                                x2lnT[:, xc:xc + 2, :],
                                start=(i == 0), stop=(i == K1 // 2 - 1),
                                perf_mode=DR,
                            )
                        nc.scalar.activation(hT[:, jj, :], ps[:], ACT.Gelu,
                                             bias=cp("fc1b", jj),
                                             scale=cp("fc1s", jj))
                    if r >= 1:
                        for j in range(6):
                            f2_mm(f2ps[j][:], j, r - 1)
                for j in range(6):
                    for i in range(15, K2 // 2):
                        f2_mm(f2ps[j][:], j, i)
                    f2_evict(f2ps[j][:], j)
                for j in (6, 7):
                    ps = tp_ps.tile([P, T], f32, tag="tp", name=f"f2{j}")
                    for i in range(K2 // 2):
                        f2_mm(ps[:], j, i)
                    f2_evict(ps[:], j)

                for ti in range(4):
                    tpm = tp_ps.tile([P, 8, P], bf16, tag="tp",
                                     name=f"tpm{ti}")
                    for ch in range(8):
                        nc.tensor.transpose(
                            tpm[:, ch, :],
                            mT[:, ch * T + ti * P:ch * T + (ti + 1) * P],
                            idb[:, :])
                    if ti % 2 == 0:
                        nc.vector.tensor_copy(
                            outt[:, ti * D:(ti + 1) * D], tpm[:])
                    else:
                        nc.scalar.activation(
                            outt[:, ti * D:(ti + 1) * D], tpm[:],
                            ACT.Identity)
                    nc.sync.dma_start(
                        out_d[ti * P:(ti + 1) * P, :],
                        outt[:, ti * D:(ti + 1) * D])

    if not nc.is_finalized():
        nc.finalize()
    return nc


def _scale_w(w):
    amax = np.abs(w).max(axis=0, keepdims=True)
    s = 2.0 ** np.round(np.log2(2.0 / np.maximum(amax, 1e-30)))
    return w * s, (1.0 / s)[0]


def _prep_w(w, comp):
    """[Din, Dout] fp32 -> ([128, kchunks, Dout] fp8 chunk-major hi(+lo),
    descale vector [Dout])."""
    din, dout = w.shape
    nch = din // P
    ws, descale = _scale_w(np.ascontiguousarray(w.astype(np.float32)))
    hi = ws.astype(F8)
    blocks = [hi]
    if comp:
        lo = (ws - hi.astype(np.float32)).astype(F8)
        blocks.append(lo)
    cols = []
    for b in blocks:
        cols.append(b.reshape(nch, P, dout).transpose(1, 0, 2))
    out = np.concatenate(cols, axis=1)  # [128, kchunks, dout]
    return np.ascontiguousarray(out), descale.astype(np.float32)


def _perm():
    """Head-minor channel permutation: new channel k*128 + h*8 + j holds
    old channel h*64 + k*8 + j, so head(partition p) = p // 8 in every
    chunk of the transposed layout."""
    p = np.empty(D, np.int64)
    for k in range(8):
        for h in range(H):
            for j in range(8):
                p[k * P + h * 8 + j] = h * HD + k * 8 + j
    return p


def _host_inputs(x, qkv_w, qkv_b, proj_w, proj_b, g1, b1, g2, b2,
                 fc1_w, fc1_b, fc2_w, fc2_b):
    scale = HD ** -0.5
    qkvw_eff = (qkv_w * g1[:, None]).astype(np.float32).copy()
    qkvb_eff = (qkv_b + b1 @ qkv_w).astype(np.float32).copy()
    qkvw_eff[:, 0:D] *= scale
    qkvb_eff[0:D] *= scale
    pm = _perm()
    for s in range(3):
        qkvw_eff[:, s * D:(s + 1) * D] = qkvw_eff[:, s * D + pm]
        qkvb_eff[s * D:(s + 1) * D] = qkvb_eff[s * D + pm]
    proj_w = np.ascontiguousarray(proj_w[pm, :])
    fc2_w = np.asarray(fc2_w, np.float32)
    bias_p = bool(np.any(proj_b))
    bias_f = bool(np.any(fc2_b))
    if bias_p:  # ones-chunk pair: extra moving chunk of 1s picks up b/128
        proj_w = np.vstack([proj_w, np.tile(proj_b[None, :] / P, (P, 1)),
                            np.zeros((P, D), np.float32)])
    if bias_f:
        fc2_w = np.vstack([fc2_w, np.tile(fc2_b[None, :] / P, (P, 1)),
                           np.zeros((P, D), np.float32)])
    fc1w_eff = (fc1_w * g2[:, None]).astype(np.float32)
    fc1b_eff = (fc1_b + b2 @ fc1_w).astype(np.float32)

    qkvw_p, qkvs_v = _prep_w(qkvw_eff, COMP["qkv"])
    projw_p, projs_v = _prep_w(proj_w.astype(np.float32), COMP["proj"])
    fc1w_p, fc1s_v = _prep_w(fc1w_eff, COMP["fc1"])
    fc2w_p, fc2s_v = _prep_w(fc2_w.astype(np.float32), COMP["fc2"])

    cpak = np.zeros((P, CPAK_W), np.float32)

    def setc(name, vec, n):
        cpak[:, _C[name]:_C[name] + n] = vec.reshape(n, P).T

    setc("qkvb", qkvb_eff, 24)
    setc("qkvs", qkvs_v, 24)
    setc("projb", proj_b.astype(np.float32), 8)
    setc("projs", projs_v, 8)
    setc("fc1b", fc1b_eff, 32)
    setc("fc1s", fc1s_v, 32)
    setc("fc2b", fc2_b.astype(np.float32), 8)
    setc("fc2s", fc2s_v, 8)
    kv_s = qkvs_v[D:3 * D].reshape(16, P)
    kv_b = qkvb_eff[D:3 * D].reshape(16, P)
    for j in range(16):
        for c in range(2):
            cpak[:, _C["khs"] + 2 * j + c] = kv_s[j]
            cpak[:, _C["khb"] + 2 * j + c] = kv_b[j]

    bpak0 = np.zeros((P, BPAK_W), np.float32)
    bpak0[:, _B["idb"]:_B["idb"] + 128] = np.eye(P)
    hm = np.zeros((P, 8, H), np.float32)
    for c in range(P):
        for ch in range(8):
            hm[c, ch, c // 8] = 1.0
    bpak0[:, _B["hmask"]:_B["hmask"] + 128] = hm.reshape(P, 8 * H)

    common = {
        "projw": np.ascontiguousarray(projw_p.reshape(P, -1)),
        "cpak": cpak,
    }
    for b in range(4):
        common[f"qkvw{b}"] = np.ascontiguousarray(
            qkvw_p[:, :, b * 768:(b + 1) * 768].reshape(P, -1))
    for b in range(2):
        common[f"fc1w{b}"] = np.ascontiguousarray(
            fc1w_p[:, :, b * 2048:(b + 1) * 2048].reshape(P, -1))
    for b in range(2):
        common[f"fc2w{b}"] = np.ascontiguousarray(
            fc2w_p[:, :, b * 512:(b + 1) * 512].reshape(P, -1))

    in_maps = []
    for core in range(NCORE):
        b, q = divmod(core, 4)
        xm = np.ascontiguousarray(x[b, q * T:(q + 1) * T, :]).astype(BF)
        bpak = bpak0.copy()
        if q == 0:
            xhv = np.zeros((2, D), BF)
            # emk stays zero
        else:
            xhv = np.ascontiguousarray(x[b, q * T - 2:q * T, :]).astype(BF)
            bpak[0:H, _B["emk"]:_B["emk"] + 3] = 1.0
        m = dict(common)
        m["xm"] = xm
        m["xh"] = xhv
        m["bpak"] = bpak.astype(BF)
        in_maps.append(m)
    return in_maps


def kernel(**inputs) -> np.ndarray:
    from concourse.bass_utils import run_bass_kernel_spmd

    key = (bool(np.any(inputs["proj_b"])), bool(np.any(inputs["fc2_b"])))
    if key not in _CACHE:
        _CACHE[key] = _build_program(bias_p=key[0], bias_f=key[1])
    nc = _CACHE[key]
    in_maps = _host_inputs(**inputs)
    res = run_bass_kernel_spmd(nc, in_maps, list(range(NCORE)))
    outs = res.results
    full = np.zeros((2, 2048, D), np.float32)
    for core in range(NCORE):
        b, q = divmod(core, 4)
        full[b, q * T:(q + 1) * T, :] = outs[core]["out"].astype(np.float32)
    return full



# revision 41
# speedup vs baseline: 1.0761x; 1.0761x over previous
"""Trainium2 Bass kernel: LocalCausalTransformerBlock (window-3 causal attention).

Sharding: 8-way sequence-parallel. B=2 x N=2048 = 4096 tokens -> 8 chunks of
512 tokens (4 chunks per batch row). Each core gets its 512 tokens plus a
2-token halo (the preceding tokens of the same sequence) so the window-3
causal attention needs no cross-core communication. Weights are replicated.

The four big matmuls (qkv/proj/fc1/fc2) run in fp8e4m3 with DoubleRow perf
mode (contract 2x128 channels per instruction at 0.5 cycles/row). Weights are
pre-scaled per output column to a power of two near absmax~2 so e4m3's
subnormal range is never hit; the descale rides the eviction's free
multiplicative scalar slot. qkv and proj additionally carry the quantization
residual ("lo") as extra fp8 k-chunks in the same accumulation group.
Attention internals (q/k/v, softmax, AV) are bf16; LayerNorm stats, softmax
normalizer and both residual streams are fp32.

Layout: activations live "transposed" (channels on partitions, tokens on the
free axis) so matmuls contract over partitions and the +-1/+-2 token shifts
of the local attention are free-axis offsets. LayerNorm runs token-major; PE
transposes bridge the layouts, batched 8-to-a-psum-bank with one wide strided
eviction. Softmax needs no max-subtraction (window-3 scores are small): exp
runs directly on the score PSUM; a per-core multiplicative edge mask zeroes
out-of-window columns after exp. Act-table funcs are ordered
sqrt->exp->sqrt->gelu (4 loads; identity is in every set). Weight matrices
stream in as column-block DMAs so matmuls start before the full matrix
lands; small constants ride in two packed DMAs. PSUM evictions are spread
across DVE and Act (gpsimd cannot touch PSUM); Pool takes SBUF-only work.
"""

import sys

for _p in ("/opt/trn_rl_repo",):
    if _p not in sys.path:
        sys.path.insert(0, _p)

import numpy as np
import ml_dtypes

P = 128
D = 1024
H = 16
HD = 64
H3 = 3 * D
HID = 4096
T = 512            # real tokens per core
TH = T + 2         # k/v token axis with 2-token halo (halo stored first)
NCORE = 8
EPS = 1e-5
BF = ml_dtypes.bfloat16
F8 = ml_dtypes.float8_e4m3

# which weights carry the fp8 quantization residual (2x k-chunks)
COMP = {"qkv": False, "proj": False, "fc1": False, "fc2": False}

# packed f32 const columns
_C = {}
_off = 0
for _name, _w in [("qkvb", 24), ("qkvs", 24), ("projb", 8), ("projs", 8),
                  ("fc1b", 32), ("fc1s", 32), ("fc2b", 8), ("fc2s", 8),
                  ("khs", 32), ("khb", 32)]:
    _C[_name] = _off
    _off += _w
CPAK_W = _off
# packed bf16 const columns: idb, hmask, emk
_B = {"idb": 0, "hmask": 128, "emk": 256}
BPAK_W = 260

_CACHE: dict = {}


def _build_program(bias_p=False, bias_f=False):
    """bias_p/bias_f: include ones-chunks in proj/fc2 matmuls to add a
    nonzero proj_b/fc2_b (the channel-major residual fusion has no other
    slot for them). Left off when the biases are zero."""
    import concourse.bass as bass
    import concourse.tile as tile
    from concourse import bacc, mybir, bass_isa
    from contextlib import ExitStack

    f32 = mybir.dt.float32
    bf16 = mybir.dt.bfloat16
    fp8 = mybir.dt.float8e4
    ALU = mybir.AluOpType
    ACT = mybir.ActivationFunctionType
    DR = mybir.MatmulPerfMode.DoubleRow

    KQ = 16 if COMP["qkv"] else 8
    KP = (16 if COMP["proj"] else 8) + (2 if bias_p else 0)
    K1 = 16 if COMP["fc1"] else 8
    K2 = (64 if COMP["fc2"] else 32) + (2 if bias_f else 0)
    NCH_A = 8 + (2 if bias_p else 0)   # attnT chunks (+ones pair)
    NCH_H = 32 + (2 if bias_f else 0)  # hT chunks (+ones pair)

    nc = bacc.Bacc()

    xh_d = nc.declare_dram_parameter("xh", [2, D], bf16, isOutput=False)
    xm_d = nc.declare_dram_parameter("xm", [T, D], bf16, isOutput=False)
    qkvw_ds = [nc.declare_dram_parameter(f"qkvw{b}", [P, KQ * 768], fp8,
                                         isOutput=False) for b in range(4)]
    projw_d = nc.declare_dram_parameter("projw", [P, KP * D], fp8, isOutput=False)
    fc1w_ds = [nc.declare_dram_parameter(f"fc1w{b}", [P, K1 * 2048], fp8,
                                         isOutput=False) for b in range(2)]
    fc2w_ds = [nc.declare_dram_parameter(f"fc2w{b}", [P, K2 * 512], fp8,
                                         isOutput=False) for b in range(2)]
    cpak_d = nc.declare_dram_parameter("cpak", [P, CPAK_W], f32, isOutput=False)
    bpak_d = nc.declare_dram_parameter("bpak", [P, BPAK_W], bf16, isOutput=False)
    out_d = nc.declare_dram_parameter("out", [T, D], bf16, isOutput=True)
    # DRAM scratch for the probs partition-broadcast round-trip
    pw_dram = nc.dram_tensor("pwd", (H, 3 * T), bf16, kind="Internal")

    with tile.TileContext(nc) as tc, ExitStack() as ctx:
        # PSUM budget (8 banks): mm x4, sc x2 (scores <-> fc2), tp x2
        const = ctx.enter_context(tc.tile_pool(name="const", bufs=1))
        acts = ctx.enter_context(tc.tile_pool(name="acts", bufs=1))
        ln_pool = ctx.enter_context(tc.tile_pool(name="ln", bufs=2))
        tp_ps = ctx.enter_context(tc.tile_pool(name="tp_ps", bufs=2, space="PSUM"))
        mm_ps = ctx.enter_context(tc.tile_pool(name="mm_ps", bufs=4, space="PSUM"))
        sc_ps = ctx.enter_context(tc.tile_pool(name="sc_ps", bufs=2, space="PSUM"))

        bpak = const.tile([P, BPAK_W], bf16, tag="bp", name="bpak")
        cpak = const.tile([P, CPAK_W], f32, tag="cp", name="cpak")

        def cp(name, j, w=1):
            o = _C[name] + j
            return cpak[:, o:o + w]

        idb = bpak[:, _B["idb"]:_B["idb"] + 128]
        hmask = bpak[:, _B["hmask"]:_B["hmask"] + 128]
        emk = bpak[0:H, _B["emk"]:_B["emk"] + 3]

        # activations alive into the MLP phases (channel-major residual)
        xT = acts.tile([P, 8, T], bf16, tag="xT", name="xT")
        x2T = acts.tile([P, 8, T], bf16, tag="x2T", name="x2T")
        x2lnT = acts.tile([P, 8, T], fp8, tag="x2lnT", name="x2lnT")
        # weights preloaded early so their DMAs overlap earlier phases
        projw = acts.tile([P, KP, D], fp8, tag="projw", name="projw")
        fc1w = [acts.tile([P, K1, 2048], fp8, tag=f"fc1w{b}", name=f"fc1w{b}")
                for b in range(2)]
        fc2w = [acts.tile([P, K2, 512], fp8, tag=f"fc2w{b}", name=f"fc2w{b}")
                for b in range(2)]

        def layernorm_tok(src_ap, s, dstT, dst_off, pool=None, tp_tag="tp",
                          tp_pool=None, split_apply=False, evict="act"):
            """Token-major LN over s tokens -> fp8 channel-major in
            dstT[:, ch, dst_off:dst_off+s]. Stats on DVE, rstd via Act sqrt +
            DVE reciprocal, apply on Pool (optionally split Pool/DVE),
            transpose batch on PE, one wide Act eviction."""
            pool = pool or ln_pool
            tp_pool = tp_pool or tp_ps
            stat = pool.tile([s, 12], bf16, tag=f"lnstat{s}", name=f"st{s}")
            nc.vector.bn_stats(stat[:, 0:6], src_ap[:, 0:512])
            nc.vector.bn_stats(stat[:, 6:12], src_ap[:, 512:1024])
            mv = pool.tile([s, 2], f32, tag=f"lnmv{s}", name=f"mv{s}")
            nc.vector.bn_aggr(mv[:], stat[:])
            vpe = pool.tile([s, 1], f32, tag=f"lnvpe{s}", name=f"vpe{s}")
            nc.vector.tensor_scalar_add(vpe[:], mv[:, 1:2], EPS)
            std = pool.tile([s, 1], f32, tag=f"lnstd{s}", name=f"sd{s}")
            nc.scalar.activation(std[:], vpe[:], ACT.Sqrt)
            rstd = pool.tile([s, 1], f32, tag=f"lnrstd{s}", name=f"rs{s}")
            nc.vector.reciprocal(rstd[:], std[:])
            nmr = pool.tile([s, 1], f32, tag=f"lnnmr{s}", name=f"nm{s}")
            nc.vector.scalar_tensor_tensor(
                nmr[:], mv[:, 0:1], -1.0, rstd[:], ALU.mult, ALU.mult
            )
            xln = pool.tile([s, D], bf16, tag=f"lnout{s}", name=f"xo{s}")
            if split_apply:
                nc.gpsimd.tensor_scalar(xln[:, 0:512], src_ap[:, 0:512],
                                        rstd[:, 0:1], nmr[:, 0:1],
                                        ALU.mult, ALU.add)
                nc.gpsimd.tensor_scalar(xln[:, 512:1024], src_ap[:, 512:1024],
                                        rstd[:, 0:1], nmr[:, 0:1],
                                        ALU.mult, ALU.add)
            else:
                nc.gpsimd.tensor_scalar(xln[:], src_ap[:], rstd[:, 0:1],
                                        nmr[:, 0:1], ALU.mult, ALU.add)
            tpw = tp_pool.tile([P, 8, s], bf16, tag=tp_tag, name=f"tpln{s}")
            for ch in range(8):
                nc.tensor.transpose(tpw[:, ch, :], xln[:, ch * P:(ch + 1) * P],
                                    idb[0:s, 0:s])
            if evict == "act":
                nc.scalar.activation(dstT[:, :, dst_off:dst_off + s], tpw[:],
                                     ACT.Identity)
            else:
                nc.vector.tensor_copy(dstT[:, :, dst_off:dst_off + s], tpw[:])

        with tc.tile_pool(name="p1", bufs=1) as p1:
            xt = p1.tile([P, 4 * D], bf16, tag="xt", name="xt")
            xh = p1.tile([2, D], bf16, tag="xh", name="xh")
            xlnT = p1.tile([P, 8, T], fp8, tag="xlnT", name="xlnT")
            xlnTh = p1.tile([P, 8, 2], fp8, tag="xlnTh", name="xlnTh")
            qT = p1.tile([P, 8 * T], bf16, tag="qT", name="qT")
            kvT = p1.tile([P, 16, TH], bf16, tag="kvT", name="kvT")

            # SP queue: halo + x first (LN1 critical), consts, then q-half
            # of the qkv weights, projw and fc2w. Act queue: k/v-half of
            # qkv weights (needed a bit later). Pool queue: fc1w, issued
            # after the LN1 applies so they don't block them.
            nc.sync.dma_start(xh[:], xh_d[:])
            for ti in range(4):
                nc.sync.dma_start(xt[:, ti * D:(ti + 1) * D],
                                  xm_d[ti * P:(ti + 1) * P, :])
            nc.sync.dma_start(bpak[:], bpak_d[:])
            nc.sync.dma_start(cpak[:], cpak_d[:])

            with tc.tile_pool(name="p3", bufs=1) as p3:
                attnT = p3.tile([P, NCH_A, T], fp8, tag="attnT",
                                name="attnT")
                if bias_p:
                    nc.vector.memset(attnT[:, 8, :], 1.0)
                    nc.vector.memzero(attnT[:, 9, :])
                with tc.tile_pool(name="p2", bufs=1) as p2:
                    et = p2.tile([H, 3, T], bf16, tag="et", name="et")
                    with tc.tile_pool(name="wq", bufs=1) as wq_pool:
                        qkvw = []
                        for b in range(4):
                            t = wq_pool.tile([P, KQ, 768], fp8,
                                             tag=f"qkvw{b}", name=f"qkvw{b}")
                            eng = nc.sync if b < 2 else nc.scalar
                            eng.dma_start(t[:], qkvw_ds[b][:])
                            qkvw.append(t)
                        nc.sync.dma_start(projw[:], projw_d[:])

                        for b in range(2):
                            nc.sync.dma_start(fc1w[b][:], fc1w_ds[b][:])
                        for b in range(2):
                            nc.sync.dma_start(fc2w[b][:], fc2w_ds[b][:])

                        # ---- LN1 (halo first: xh lands first) ----
                        layernorm_tok(xh[:], 2, xlnTh, 0)
                        for ti in range(4):
                            layernorm_tok(xt[:, ti * D:(ti + 1) * D], P,
                                          xlnT, ti * P)

                        # channel-major raw x for the residual stream;
                        # issued after LN1 so its Act evictions don't
                        # delay the xlnT chain (needed only at proj time)
                        for ti in range(4):
                            tpx = tp_ps.tile([P, 8, P], bf16, tag="tp",
                                             name=f"tpx{ti}")
                            for ch in range(8):
                                nc.tensor.transpose(
                                    tpx[:, ch, :],
                                    xt[:, ti * D + ch * P:
                                       ti * D + (ch + 1) * P],
                                    idb[:, :])
                            nc.scalar.activation(
                                xT[:, :, ti * P:(ti + 1) * P], tpx[:],
                                ACT.Identity)

                        # ---- QKV ----
                        # halo k/v columns: one psum tile = 16 blocks x 2 cols
                        ph = tp_ps.tile([P, 8, 4], f32, tag="tp", name="ph")
                        for j in range(16):
                            col = D + j * P
                            wt = qkvw[col // 768]
                            wo = col % 768
                            for i in range(KQ // 2):
                                xc = (2 * i) % 8
                                nc.tensor.matmul(
                                    ph[:, j // 2, (j % 2) * 2:(j % 2) * 2 + 2],
                                    wt[:, 2 * i:2 * i + 2, wo:wo + P],
                                    xlnTh[:, xc:xc + 2, :],
                                    start=(i == 0), stop=(i == KQ // 2 - 1),
                                    perf_mode=DR,
                                )
                        pht = ln_pool.tile([P, 32], f32, tag="pht", name="pht")
                        nc.vector.tensor_mul(pht[:], ph[:, :, :],
                                             cp("khs", 0, 32))
                        for j in range(16):
                            nc.gpsimd.tensor_add(
                                kvT[:, j, 0:2], pht[:, 2 * j:2 * j + 2],
                                cp("khb", 2 * j, 2))

                        def qkv_tile(j):
                            wt = qkvw[j // 6]
                            wo = (j % 6) * P
                            ps = mm_ps.tile([P, T], f32, tag="mm",
                                            name=f"qkv{j}")
                            for i in range(KQ // 2):
                                xc = (2 * i) % 8
                                nc.tensor.matmul(
                                    ps[:], wt[:, 2 * i:2 * i + 2, wo:wo + P],
                                    xlnT[:, xc:xc + 2, :],
                                    start=(i == 0), stop=(i == KQ // 2 - 1),
                                    perf_mode=DR,
                                )
                            if j < 8:
                                dst = qT[:, j * T:(j + 1) * T]
                            else:
                                dst = kvT[:, j - 8, 2:TH]
                            if j % 2 == 0:
                                nc.vector.tensor_scalar(
                                    dst, ps[:], cp("qkvs", j), cp("qkvb", j),
                                    ALU.mult, ALU.add)
                            else:
                                nc.scalar.activation(dst, ps[:], ACT.Identity,
                                                     bias=cp("qkvb", j),
                                                     scale=cp("qkvs", j))

                        for j in range(16):      # q then k
                            qkv_tile(j)
                        # scores overlap the v-tile matmuls below
                        for w in range(3):
                            e = p2.tile([P, 4, T], bf16, tag="e", bufs=2,
                                        name=f"e{w}")
                            e2 = p2.tile([P, 4, T], bf16, tag="e", bufs=2,
                                         name=f"e2{w}")
                            nc.vector.tensor_mul(
                                e[:], qT[:, 0:4 * T],
                                kvT[:, 0:4, 2 - w:2 - w + T])
                            nc.vector.tensor_mul(
                                e2[:], qT[:, 4 * T:8 * T],
                                kvT[:, 4:8, 2 - w:2 - w + T])
                            sc = sc_ps.tile([H, T], f32, tag="sc",
                                            name=f"sc{w}")
                            for ch in range(8):
                                esrc = e if ch < 4 else e2
                                nc.tensor.matmul(
                                    sc[:], hmask[:, ch * H:(ch + 1) * H],
                                    esrc[:, ch % 4, :],
                                    start=(ch == 0), stop=(ch == 7),
                                )
                            nc.scalar.activation(et[:, w, :], sc[:], ACT.Exp)
                        # preload the sqrt act table for LN2 while Act
                        # has slack (identity is in every table)
                        scr = ln_pool.tile([P, 1], f32, tag="scr", name="scr")
                        nc.scalar.activation(scr[:], cp("qkvs", 0), ACT.Sqrt)
                        # ---- softmax (before the v evictions so pw is
                        # ready when the PE reaches the bc matmuls) ----
                        nc.gpsimd.tensor_mul(et[:, 1, 0:1], et[:, 1, 0:1],
                                             emk[:, 0:1])
                        nc.gpsimd.tensor_mul(et[:, 2, 0:2], et[:, 2, 0:2],
                                             emk[:, 1:3])
                        z0 = p2.tile([H, T], bf16, tag="z0", name="z0")
                        z1 = p2.tile([H, T], bf16, tag="z1", name="z1")
                        rz = p2.tile([H, T], bf16, tag="z0", name="rz")
                        nc.gpsimd.tensor_add(z0[:], et[:, 0, :], et[:, 1, :])
                        nc.gpsimd.tensor_add(z1[:], z0[:], et[:, 2, :])
                        with nc.allow_low_precision(reason="softmax bf16"):
                            nc.vector.reciprocal(rz[:], z1[:])
                        for w in range(3):
                            nc.vector.tensor_mul(et[:, w, :], et[:, w, :],
                                                 rz[:])
                        # probs partition-broadcast: SBUF -> DRAM, then ONE
                        # stride-0 DMA fans head rows out. Channels are
                        # host-permuted head-minor (head = partition // 8,
                        # identical in every chunk), so a single [128, 3T]
                        # tile serves all 8 chunks.
                        nc.scalar.dma_start(pw_dram[:], et[:, :, :])
                        bcs = p2.tile([P, 3, T], bf16, tag="bcs", name="bcs")
                        import concourse.ap as cap
                        src = cap.AP(pw_dram, 0,
                                     [[3 * T, H], [0, 8], [1, 3 * T]])
                        nc.scalar.dma_start(bcs[:], src)
                        for j in range(16, 24):  # v
                            qkv_tile(j)

                        for chp in range(4):  # chunk pairs, fully streamed
                            ch = 2 * chp
                            avs = []
                            for w in range(3):
                                av = p2.tile([P, 2, T], bf16, tag="av",
                                             bufs=4, name=f"av{chp}_{w}")
                                for c in range(2):
                                    nc.vector.tensor_mul(
                                        av[:, c, :], bcs[:, w, :],
                                        kvT[:, 8 + ch + c,
                                            2 - w:2 - w + T],
                                    )
                                avs.append(av)
                            av01 = p2.tile([P, 2, T], bf16, tag="av01",
                                           bufs=2, name=f"av01_{chp}")
                            eng = nc.vector if chp == 3 else nc.gpsimd
                            eng.tensor_add(av01[:], avs[0][:], avs[1][:])
                            eng.tensor_add(attnT[:, ch:ch + 2, :],
                                           av01[:], avs[2][:])

                # ---- proj + residual 1 + LN2 (all channel-major) ----
                with tc.tile_pool(name="p5", bufs=1) as p5:
                    # 8 concurrent psum groups streaming over attnT pairs
                    pjps = {}
                    for j in range(8):
                        pool, tag = [(sc_ps, "sc"), (mm_ps, "mm"),
                                     (tp_ps, "tp")][0 if j < 2 else
                                                    (1 if j < 6 else 2)]
                        pjps[j] = pool.tile([P, T], f32, tag=tag,
                                            name=f"pj{j}")
                    for i in range(KP // 2):
                        for j in range(8):
                            nc.tensor.matmul(
                                pjps[j][:], projw[:, 2 * i:2 * i + 2,
                                                  j * P:(j + 1) * P],
                                attnT[:, 2 * i:2 * i + 2, :],
                                start=(i == 0), stop=(i == KP // 2 - 1),
                                perf_mode=DR,
                            )
                    # fused evict + scale + residual: x2T = pj*s + xT.
                    # Even groups via DVE STT; odd groups via Act evict +
                    # Pool add so three engines share the drain.
                    ytmp = p5.tile([P, 4, T], bf16, tag="ytmp", name="ytmp")
                    for j in range(8):
                        if j % 2 == 0:
                            nc.vector.scalar_tensor_tensor(
                                x2T[:, j, :], pjps[j][:], cp("projs", j),
                                xT[:, j, :], ALU.mult, ALU.add)
                        else:
                            nc.scalar.activation(ytmp[:, j // 2, :],
                                                 pjps[j][:], ACT.Identity,
                                                 scale=cp("projs", j))
                            nc.gpsimd.tensor_add(x2T[:, j, :],
                                                 ytmp[:, j // 2, :],
                                                 xT[:, j, :])
                    # LN2 stats channel-major: pairwise folds + gpsimd
                    # partition all-reduce give per-token sums broadcast
                    # to every partition; row math runs on those tiles.
                    # Processed in two token halves so the apply (and
                    # fc1's input) is ready sooner.
                    sq = p5.tile([P, 8, T], bf16, tag="sq", name="sq")
                    f1 = p5.tile([P, 4, T], bf16, tag="f1", name="f1")
                    g1t = p5.tile([P, 4, T], bf16, tag="g1t", name="g1t")
                    f2t = p5.tile([P, 2, T], bf16, tag="f2t", name="f2t")
                    g2t = p5.tile([P, 2, T], bf16, tag="g2t", name="g2t")
                    f3 = p5.tile([P, T], bf16, tag="f3", name="f3")
                    g3 = p5.tile([P, T], bf16, tag="g3", name="g3")
                    ars = p5.tile([P, T], f32, tag="ars", name="ars")
                    arq = p5.tile([P, T], f32, tag="arq", name="arq")
                    mu = p5.tile([P, T], bf16, tag="mu", name="mu")
                    e2n = p5.tile([P, T], bf16, tag="e2n", name="e2n")
                    mu2 = p5.tile([P, T], bf16, tag="mu2", name="mu2")
                    var = p5.tile([P, T], bf16, tag="var", name="var")
                    stdt = p5.tile([P, T], bf16, tag="stdt", name="stdt")
                    rstd = p5.tile([P, T], bf16, tag="rstdb", name="rstdb")
                    mrs = p5.tile([P, T], bf16, tag="mrs", name="mrs")
                    t1 = p5.tile([P, 8, T], bf16, tag="t1", name="t1")
                    TH2 = T // 2
                    for h2 in range(2):
                        s = slice(h2 * TH2, (h2 + 1) * TH2)
                        nc.vector.tensor_mul(sq[:, :, s], x2T[:, :, s],
                                             x2T[:, :, s])
                        nc.vector.tensor_add(f1[:, :, s], x2T[:, 0:4, s],
                                             x2T[:, 4:8, s])
                        nc.gpsimd.tensor_add(g1t[:, :, s], sq[:, 0:4, s],
                                             sq[:, 4:8, s])
                        nc.vector.tensor_add(f2t[:, :, s], f1[:, 0:2, s],
                                             f1[:, 2:4, s])
                        nc.gpsimd.tensor_add(g2t[:, :, s], g1t[:, 0:2, s],
                                             g1t[:, 2:4, s])
                        nc.vector.tensor_add(f3[:, s], f2t[:, 0, s],
                                             f2t[:, 1, s])
                        nc.gpsimd.tensor_add(g3[:, s], g2t[:, 0, s],
                                             g2t[:, 1, s])
                        nc.gpsimd.partition_all_reduce(
                            ars[:, s], f3[:, s], P, bass_isa.ReduceOp.add)
                        nc.gpsimd.partition_all_reduce(
                            arq[:, s], g3[:, s], P, bass_isa.ReduceOp.add)
                        nc.vector.tensor_scalar_mul(mu[:, s], ars[:, s],
                                                    1.0 / D)
                        nc.vector.tensor_scalar(e2n[:, s], arq[:, s],
                                                1.0 / D, EPS,
                                                ALU.mult, ALU.add)
                        nc.gpsimd.tensor_mul(mu2[:, s], mu[:, s], mu[:, s])
                        nc.vector.tensor_sub(var[:, s], e2n[:, s],
                                             mu2[:, s])
                        nc.scalar.activation(stdt[:, s], var[:, s],
                                             ACT.Sqrt)
                        with nc.allow_low_precision(reason="ln2 bf16"):
                            nc.vector.reciprocal(rstd[:, s], stdt[:, s])
                        nc.gpsimd.tensor_mul(mrs[:, s], mu[:, s],
                                             rstd[:, s])
                        for ch in range(8):
                            ea = nc.vector if ch % 2 == 0 else nc.gpsimd
                            eb = nc.gpsimd if ch % 2 == 0 else nc.vector
                            ea.tensor_mul(t1[:, ch, s], x2T[:, ch, s],
                                          rstd[:, s])
                            eb.tensor_sub(x2lnT[:, ch, s], t1[:, ch, s],
                                          mrs[:, s])
                    scr2 = ln_pool.tile([P, 1], f32, tag="scr", name="scr2")
                    nc.scalar.activation(scr2[:], cp("qkvs", 0), ACT.Gelu)

        # ---- MLP fc1 + gelu, fc2 + residual 2 + store ----
        # fc1 tiles rotate on tp_ps; fc2 keeps 6 psum groups live on
        # mm_ps+sc_ps for the whole phase, its i-step lagging the fc1
        # round that produced those hT chunks by one round so the
        # in-order PE queue never stalls on a gelu eviction.
        with tc.tile_pool(name="w1", bufs=1) as w1_pool:
                outt = w1_pool.tile([P, 4 * D], bf16, tag="outt", name="outt")
                mT = w1_pool.tile([P, 8 * T], bf16, tag="mT", name="mT")
                hT = w1_pool.tile([P, NCH_H, T], fp8, tag="hT", name="hT")
                if bias_f:
                    nc.vector.memset(hT[:, 32, :], 1.0)
                    nc.vector.memzero(hT[:, 33, :])

                def f2_mm(ps, j, i):
                    wt = fc2w[j // 4]
                    wo = (j % 4) * P
                    nc.tensor.matmul(
                        ps[:], wt[:, 2 * i:2 * i + 2, wo:wo + P],
                        hT[:, 2 * i:2 * i + 2, :],
                        start=(i == 0), stop=(i == K2 // 2 - 1),
                        perf_mode=DR,
                    )

                # fused evict + scale + residual: out3 = f2*s + x2T.
                # Even groups: DVE STT from psum; odd: Act evict + Pool add.
                mtmp = w1_pool.tile([P, 4, T], bf16, tag="mtmp", name="mtmp")

                def f2_evict(ps, j):
                    if j % 2 == 0:
                        nc.vector.scalar_tensor_tensor(
                            mT[:, j * T:(j + 1) * T], ps[:], cp("fc2s", j),
                            x2T[:, j, :], ALU.mult, ALU.add)
                    else:
                        nc.scalar.activation(mtmp[:, j // 2, :], ps[:],
                                             ACT.Identity,
                                             scale=cp("fc2s", j))
                        nc.gpsimd.tensor_add(mT[:, j * T:(j + 1) * T],
                                             mtmp[:, j // 2, :],
                                             x2T[:, j, :])

                f2ps = {}
                for j in range(6):
                    pool = sc_ps if j < 2 else mm_ps
                    f2ps[j] = pool.tile([P, T], f32,
                                        tag="sc" if j < 2 else "mm",
                                        name=f"f2{j}")

                for r in range(16):
                    for jj in (2 * r, 2 * r + 1):
                        wt = fc1w[jj // 16]
                        wo = (jj % 16) * P
                        ps = tp_ps.tile([P, T], f32, tag="tp", name=f"f1{jj}")
                        for i in range(K1 // 2):
                            xc = (2 * i) % 8
                            nc.tensor.matmul(
                                ps[:], wt[:, 2 * i:2 * i + 2, wo:wo + P],
                                x2lnT[:, xc:xc + 2, :],
                                start=(i == 0), stop=(i == K1 // 2 - 1),
                                perf_mode=DR,
                            )
                        nc.scalar.activation(hT[:, jj, :], ps[:], ACT.Gelu,
                                             bias=cp("fc1b", jj),
                                             scale=cp("fc1s", jj))
                    if r >= 1:
                        for j in range(6):
                            f2_mm(f2ps[j][:], j, r - 1)
                for j in range(6):
                    for i in range(15, K2 // 2):
                        f2_mm(f2ps[j][:], j, i)
                    f2_evict(f2ps[j][:], j)
                for j in (6, 7):
                    ps = tp_ps.tile([P, T], f32, tag="tp", name=f"f2{j}")
                    for i in range(K2 // 2):
                        f2_mm(ps[:], j, i)
                    f2_evict(ps[:], j)

                for ti in range(4):
                    tpm = tp_ps.tile([P, 8, P], bf16, tag="tp",
                                     name=f"tpm{ti}")
                    for ch in range(8):
                        nc.tensor.transpose(
                            tpm[:, ch, :],
                            mT[:, ch * T + ti * P:ch * T + (ti + 1) * P],
                            idb[:, :])
                    if ti % 2 == 0:
                        nc.vector.tensor_copy(
                            outt[:, ti * D:(ti + 1) * D], tpm[:])
                    else:
                        nc.scalar.activation(
                            outt[:, ti * D:(ti + 1) * D], tpm[:],
                            ACT.Identity)
                    nc.sync.dma_start(
                        out_d[ti * P:(ti + 1) * P, :],
                        outt[:, ti * D:(ti + 1) * D])

    if not nc.is_finalized():
        nc.finalize()
    return nc


def _scale_w(w):
    amax = np.abs(w).max(axis=0, keepdims=True)
    s = 2.0 ** np.round(np.log2(2.0 / np.maximum(amax, 1e-30)))
    return w * s, (1.0 / s)[0]


def _prep_w(w, comp):
    """[Din, Dout] fp32 -> ([128, kchunks, Dout] fp8 chunk-major hi(+lo),
    descale vector [Dout])."""
    din, dout = w.shape
    nch = din // P
    ws, descale = _scale_w(np.ascontiguousarray(w.astype(np.float32)))
    hi = ws.astype(F8)
    blocks = [hi]
    if comp:
        lo = (ws - hi.astype(np.float32)).astype(F8)
        blocks.append(lo)
    cols = []
    for b in blocks:
        cols.append(b.reshape(nch, P, dout).transpose(1, 0, 2))
    out = np.concatenate(cols, axis=1)  # [128, kchunks, dout]
    return np.ascontiguousarray(out), descale.astype(np.float32)


def _perm():
    """Head-minor channel permutation: new channel k*128 + h*8 + j holds
    old channel h*64 + k*8 + j, so head(partition p) = p // 8 in every
    chunk of the transposed layout."""
    p = np.empty(D, np.int64)
    for k in range(8):
        for h in range(H):
            for j in range(8):
                p[k * P + h * 8 + j] = h * HD + k * 8 + j
    return p


def _host_inputs(x, qkv_w, qkv_b, proj_w, proj_b, g1, b1, g2, b2,
                 fc1_w, fc1_b, fc2_w, fc2_b):
    scale = HD ** -0.5
    qkvw_eff = (qkv_w * g1[:, None]).astype(np.float32).copy()
    qkvb_eff = (qkv_b + b1 @ qkv_w).astype(np.float32).copy()
    qkvw_eff[:, 0:D] *= scale
    qkvb_eff[0:D] *= scale
    pm = _perm()
    for s in range(3):
        qkvw_eff[:, s * D:(s + 1) * D] = qkvw_eff[:, s * D + pm]
        qkvb_eff[s * D:(s + 1) * D] = qkvb_eff[s * D + pm]
    proj_w = np.ascontiguousarray(proj_w[pm, :])
    fc2_w = np.asarray(fc2_w, np.float32)
    bias_p = bool(np.any(proj_b))
    bias_f = bool(np.any(fc2_b))
    if bias_p:  # ones-chunk pair: extra moving chunk of 1s picks up b/128
        proj_w = np.vstack([proj_w, np.tile(proj_b[None, :] / P, (P, 1)),
                            np.zeros((P, D), np.float32)])
    if bias_f:
        fc2_w = np.vstack([fc2_w, np.tile(fc2_b[None, :] / P, (P, 1)),
                           np.zeros((P, D), np.float32)])
    fc1w_eff = (fc1_w * g2[:, None]).astype(np.float32)
    fc1b_eff = (fc1_b + b2 @ fc1_w).astype(np.float32)

    qkvw_p, qkvs_v = _prep_w(qkvw_eff, COMP["qkv"])
    projw_p, projs_v = _prep_w(proj_w.astype(np.float32), COMP["proj"])
    fc1w_p, fc1s_v = _prep_w(fc1w_eff, COMP["fc1"])
    fc2w_p, fc2s_v = _prep_w(fc2_w.astype(np.float32), COMP["fc2"])

    cpak = np.zeros((P, CPAK_W), np.float32)

    def setc(name, vec, n):
        cpak[:, _C[name]:_C[name] + n] = vec.reshape(n, P).T

    setc("qkvb", qkvb_eff, 24)
    setc("qkvs", qkvs_v, 24)
    setc("projb", proj_b.astype(np.float32), 8)
    setc("projs", projs_v, 8)
    setc("fc1b", fc1b_eff, 32)
    setc("fc1s", fc1s_v, 32)
    setc("fc2b", fc2_b.astype(np.float32), 8)
    setc("fc2s", fc2s_v, 8)
    kv_s = qkvs_v[D:3 * D].reshape(16, P)
    kv_b = qkvb_eff[D:3 * D].reshape(16, P)
    for j in range(16):
        for c in range(2):
            cpak[:, _C["khs"] + 2 * j + c] = kv_s[j]
            cpak[:, _C["khb"] + 2 * j + c] = kv_b[j]

    bpak0 = np.zeros((P, BPAK_W), np.float32)
    bpak0[:, _B["idb"]:_B["idb"] + 128] = np.eye(P)
    hm = np.zeros((P, 8, H), np.float32)
    for c in range(P):
        for ch in range(8):
            hm[c, ch, c // 8] = 1.0
    bpak0[:, _B["hmask"]:_B["hmask"] + 128] = hm.reshape(P, 8 * H)

    common = {
        "projw": np.ascontiguousarray(projw_p.reshape(P, -1)),
        "cpak": cpak,
    }
    for b in range(4):
        common[f"qkvw{b}"] = np.ascontiguousarray(
            qkvw_p[:, :, b * 768:(b + 1) * 768].reshape(P, -1))
    for b in range(2):
        common[f"fc1w{b}"] = np.ascontiguousarray(
            fc1w_p[:, :, b * 2048:(b + 1) * 2048].reshape(P, -1))
    for b in range(2):
        common[f"fc2w{b}"] = np.ascontiguousarray(
            fc2w_p[:, :, b * 512:(b + 1) * 512].reshape(P, -1))

    in_maps = []
    for core in range(NCORE):
        b, q = divmod(core, 4)
        xm = np.ascontiguousarray(x[b, q * T:(q + 1) * T, :]).astype(BF)
        bpak = bpak0.copy()
        if q == 0:
            xhv = np.zeros((2, D), BF)
            # emk stays zero
        else:
            xhv = np.ascontiguousarray(x[b, q * T - 2:q * T, :]).astype(BF)
            bpak[0:H, _B["emk"]:_B["emk"] + 3] = 1.0
        m = dict(common)
        m["xm"] = xm
        m["xh"] = xhv
        m["bpak"] = bpak.astype(BF)
        in_maps.append(m)
    return in_maps


def kernel(**inputs) -> np.ndarray:
    from concourse.bass_utils import run_bass_kernel_spmd

    key = (bool(np.any(inputs["proj_b"])), bool(np.any(inputs["fc2_b"])))
    if key not in _CACHE:
        _CACHE[key] = _build_program(bias_p=key[0], bias_f=key[1])
    nc = _CACHE[key]
    in_maps = _host_inputs(**inputs)
    res = run_bass_kernel_spmd(nc, in_maps, list(range(NCORE)))
    outs = res.results
    full = np.zeros((2, 2048, D), np.float32)
    for core in range(NCORE):
        b, q = divmod(core, 4)
        full[b, q * T:(q + 1) * T, :] = outs[core]["out"].astype(np.float32)
    return full



# revision 43
# speedup vs baseline: 1.1259x; 1.0463x over previous
"""Trainium2 Bass kernel: LocalCausalTransformerBlock (window-3 causal attention).

Sharding: 8-way sequence-parallel. B=2 x N=2048 = 4096 tokens -> 8 chunks of
512 tokens (4 chunks per batch row). Each core gets its 512 tokens plus a
2-token halo (the preceding tokens of the same sequence, prepended host-side)
so the window-3 causal attention needs no cross-core communication. Weights
are replicated.

Everything on-device is channel-major (channels on partitions, tokens on the
free axis): the host hands x pre-transposed with the halo prepended and
un-transposes the output, so the PE does only matmuls - no transposes at all.
Both layernorms run channel-major: pairwise chunk folds + a gpsimd partition
all-reduce produce per-token sums broadcast to every partition, the rstd/mu
row math runs on those broadcast tiles in bf16, and the apply is two
tensor-tensor passes (in two token halves so downstream consumers start
sooner). LN gammas fold into the following matmul's weights host-side.

The four big matmuls (qkv/proj/fc1/fc2) run in fp8e4m3 with DoubleRow perf
mode (0.5 cycles/row). Weights are pre-scaled per output column to a power of
two near absmax~2; the descale rides the evictions. The proj and fc2
evictions are scalar_tensor_tensor ops that fuse descale + residual add in
one instruction (nonzero proj/fc2 biases, if ever present, ride an extra
ones-chunk in the contraction). k/v channels are host-permuted head-minor
(head = partition // 8, identical in every chunk) so the softmax probs
broadcast from 16 head rows to 128 partitions is a single stride-0 DMA via a
DRAM round-trip. Softmax needs no max-subtraction (window-3 scores are
small). DMAs are spread over the SP/Activation/Pool queues, which the cost
model executes concurrently.
"""

import sys

for _p in ("/opt/trn_rl_repo",):
    if _p not in sys.path:
        sys.path.insert(0, _p)

import numpy as np
import ml_dtypes

P = 128
D = 1024
H = 16
HD = 64
H3 = 3 * D
HID = 4096
T = 512            # real tokens per core
TH = T + 2         # token axis with 2-token halo (halo stored first)
NCORE = 8
EPS = 1e-5
BF = ml_dtypes.bfloat16
F8 = ml_dtypes.float8_e4m3

# which weights carry the fp8 quantization residual (2x k-chunks)
COMP = {"qkv": False, "proj": False, "fc1": False, "fc2": False}

# packed f32 const columns
_C = {}
_off = 0
for _name, _w in [("qkvb", 24), ("qkvs", 24), ("projs", 8),
                  ("fc1b", 32), ("fc1s", 32), ("fc2s", 8),
                  ("khs", 32), ("khb", 32)]:
    _C[_name] = _off
    _off += _w
CPAK_W = _off
# packed bf16 const columns: hmask, emk
_B = {"hmask": 0, "emk": 128}
BPAK_W = 131

_CACHE: dict = {}


def _build_program(bias_p=False, bias_f=False):
    """bias_p/bias_f: include ones-chunks in proj/fc2 matmuls to add a
    nonzero proj_b/fc2_b (the fused residual evictions have no other slot
    for them). Left off when the biases are zero."""
    import concourse.bass as bass
    import concourse.tile as tile
    import concourse.ap as cap
    from concourse import bacc, mybir, bass_isa
    from contextlib import ExitStack

    f32 = mybir.dt.float32
    bf16 = mybir.dt.bfloat16
    fp8 = mybir.dt.float8e4
    ALU = mybir.AluOpType
    ACT = mybir.ActivationFunctionType
    DR = mybir.MatmulPerfMode.DoubleRow

    KQ = 16 if COMP["qkv"] else 8
    KP = (16 if COMP["proj"] else 8) + (2 if bias_p else 0)
    K1 = 16 if COMP["fc1"] else 8
    K2 = (64 if COMP["fc2"] else 32) + (2 if bias_f else 0)
    NCH_A = 8 + (2 if bias_p else 0)   # attnT chunks (+ones pair)
    NCH_H = 32 + (2 if bias_f else 0)  # hT chunks (+ones pair)

    nc = bacc.Bacc()

    xmT_d = nc.declare_dram_parameter("xmT", [P, 8 * TH], bf16,
                                      isOutput=False)
    qkvw_ds = [nc.declare_dram_parameter(f"qkvw{b}", [P, KQ * 768], fp8,
                                         isOutput=False) for b in range(4)]
    projw_d = nc.declare_dram_parameter("projw", [P, KP * D], fp8,
                                        isOutput=False)
    fc1w_ds = [nc.declare_dram_parameter(f"fc1w{b}", [P, K1 * 2048], fp8,
                                         isOutput=False) for b in range(2)]
    fc2w_ds = [nc.declare_dram_parameter(f"fc2w{b}", [P, K2 * 512], fp8,
                                         isOutput=False) for b in range(2)]
    cpak_d = nc.declare_dram_parameter("cpak", [P, CPAK_W], f32,
                                       isOutput=False)
    bpak_d = nc.declare_dram_parameter("bpak", [P, BPAK_W], bf16,
                                       isOutput=False)
    out_d = nc.declare_dram_parameter("out", [D, T], bf16, isOutput=True)
    # DRAM scratch for the probs partition-broadcast round-trip
    pw_dram = nc.dram_tensor("pwd", (H, 3 * T), bf16, kind="Internal")

    with tile.TileContext(nc) as tc, ExitStack() as ctx:
        # PSUM budget (8 banks): mm x4, sc x2, tp x2
        const = ctx.enter_context(tc.tile_pool(name="const", bufs=1))
        acts = ctx.enter_context(tc.tile_pool(name="acts", bufs=1))
        ln_pool = ctx.enter_context(tc.tile_pool(name="ln", bufs=2))
        tp_ps = ctx.enter_context(tc.tile_pool(name="tp_ps", bufs=2,
                                               space="PSUM"))
        mm_ps = ctx.enter_context(tc.tile_pool(name="mm_ps", bufs=4,
                                               space="PSUM"))
        sc_ps = ctx.enter_context(tc.tile_pool(name="sc_ps", bufs=2,
                                               space="PSUM"))

        bpak = const.tile([P, BPAK_W], bf16, tag="bp", name="bpak")
        cpak = const.tile([P, CPAK_W], f32, tag="cp", name="cpak")

        def cp(name, j, w=1):
            o = _C[name] + j
            return cpak[:, o:o + w]

        hmask = bpak[:, _B["hmask"]:_B["hmask"] + 128]
        emk = bpak[0:H, _B["emk"]:_B["emk"] + 3]

        # activations alive into the MLP phases (channel-major residual)
        x2T = acts.tile([P, 8, T], bf16, tag="x2T", name="x2T")
        x2lnT = acts.tile([P, 8, T], fp8, tag="x2lnT", name="x2lnT")
        # weights preloaded early so their DMAs overlap earlier phases
        projw = acts.tile([P, KP, D], fp8, tag="projw", name="projw")
        fc1w = [acts.tile([P, K1, 2048], fp8, tag=f"fc1w{b}",
                          name=f"fc1w{b}") for b in range(2)]
        fc2w = [acts.tile([P, K2, 512], fp8, tag=f"fc2w{b}",
                          name=f"fc2w{b}") for b in range(2)]

        def ln_chan(src, ntok, dst, pool, pfx):
            """Channel-major layernorm: src [P, 8, ntok] bf16 ->
            dst [P, 8, ntok] fp8, per-token stats over the 1024 channels.
            Pairwise chunk folds (DVE sum-path, Pool square-path), gpsimd
            partition all-reduce broadcasts the per-token sums, bf16 row
            math, two-op apply; all pipelined in two token halves."""
            def t_(shape, tag):
                return pool.tile(shape, bf16, tag=pfx + tag,
                                 name=pfx + tag)

            sq = t_([P, 8, ntok], "sq")   # also reused as t1 space
            f1 = t_([P, 4, ntok], "f1")
            gq1 = t_([P, 4, ntok], "gq1")
            ars = t_([P, ntok], "ars")
            arq = t_([P, ntok], "arq")
            mu = t_([P, ntok], "mu")
            e2n = t_([P, ntok], "e2n")
            mu2 = t_([P, ntok], "mu2")
            stdt = t_([P, ntok], "stdt")
            nh = ntok // 2
            for h2 in range(2):
                s = slice(h2 * nh, (h2 + 1) * nh)
                nc.vector.tensor_mul(sq[:, :, s], src[:, :, s], src[:, :, s])
                nc.vector.tensor_add(f1[:, :, s], src[:, 0:4, s],
                                     src[:, 4:8, s])
                nc.gpsimd.tensor_add(gq1[:, :, s], sq[:, 0:4, s],
                                     sq[:, 4:8, s])
                nc.vector.tensor_add(f1[:, 0:2, s], f1[:, 0:2, s],
                                     f1[:, 2:4, s])
                nc.gpsimd.tensor_add(gq1[:, 0:2, s], gq1[:, 0:2, s],
                                     gq1[:, 2:4, s])
                nc.vector.tensor_add(f1[:, 0, s], f1[:, 0, s], f1[:, 1, s])
                nc.gpsimd.tensor_add(gq1[:, 0, s], gq1[:, 0, s],
                                     gq1[:, 1, s])
                nc.gpsimd.partition_all_reduce(ars[:, s], f1[:, 0, s], P,
                                               bass_isa.ReduceOp.add)
                nc.gpsimd.partition_all_reduce(arq[:, s], gq1[:, 0, s], P,
                                               bass_isa.ReduceOp.add)
                nc.vector.tensor_scalar_mul(mu[:, s], ars[:, s], 1.0 / D)
                nc.vector.tensor_scalar(e2n[:, s], arq[:, s], 1.0 / D,
                                        EPS, ALU.mult, ALU.add)
                nc.gpsimd.tensor_mul(mu2[:, s], mu[:, s], mu[:, s])
                nc.vector.tensor_sub(e2n[:, s], e2n[:, s], mu2[:, s])
                nc.scalar.activation(stdt[:, s], e2n[:, s], ACT.Sqrt)
                with nc.allow_low_precision(reason="ln bf16 rows"):
                    nc.vector.reciprocal(stdt[:, s], stdt[:, s])
                nc.gpsimd.tensor_mul(mu[:, s], mu[:, s], stdt[:, s])
                # stdt now holds rstd; mu holds mu*rstd
                for ch in range(8):
                    ea = nc.vector if ch % 2 == 0 else nc.gpsimd
                    eb = nc.gpsimd if ch % 2 == 0 else nc.vector
                    ea.tensor_mul(sq[:, ch, s], src[:, ch, s], stdt[:, s])
                    eb.tensor_sub(dst[:, ch, s], sq[:, ch, s], mu[:, s])

        with tc.tile_pool(name="p1", bufs=1) as p1:
            xmT = p1.tile([P, 8, TH], bf16, tag="xmT", name="xmT")
            xlnT = p1.tile([P, 8, TH], fp8, tag="xlnT", name="xlnT")
            qT = p1.tile([P, 8 * T], bf16, tag="qT", name="qT")
            kvT = p1.tile([P, 16, TH], bf16, tag="kvT", name="kvT")
            xT = xmT[:, :, 2:TH]  # residual view (real tokens)

            # SP queue: x first (LN1 critical), consts, q-half of qkv
            # weights, then projw/fc1w/fc2w. Act queue: k/v-half of qkv.
            nc.sync.dma_start(xmT[:], xmT_d[:])
            nc.sync.dma_start(bpak[:], bpak_d[:])
            nc.sync.dma_start(cpak[:], cpak_d[:])

            with tc.tile_pool(name="p3", bufs=1) as p3:
                attnT = p3.tile([P, NCH_A, T], fp8, tag="attnT",
                                name="attnT")
                if bias_p:
                    nc.vector.memset(attnT[:, 8, :], 1.0)
                    nc.vector.memzero(attnT[:, 9, :])
                with tc.tile_pool(name="p2", bufs=1) as p2:
                    et = p2.tile([H, 3, T], bf16, tag="et", name="et")
                    with tc.tile_pool(name="wq", bufs=1) as wq_pool:
                        qkvw = []
                        for b in range(4):
                            t = wq_pool.tile([P, KQ, 768], fp8,
                                             tag=f"qkvw{b}", name=f"qkvw{b}")
                            eng = nc.sync if b < 2 else nc.scalar
                            eng.dma_start(t[:], qkvw_ds[b][:])
                            qkvw.append(t)
                        nc.sync.dma_start(projw[:], projw_d[:])
                        for b in range(2):
                            nc.sync.dma_start(fc1w[b][:], fc1w_ds[b][:])
                        for b in range(2):
                            nc.sync.dma_start(fc2w[b][:], fc2w_ds[b][:])

                        # ---- LN1 (channel-major, incl. halo columns) ----
                        with tc.tile_pool(name="lnp", bufs=1) as lnp:
                            ln_chan(xmT[:, :, :], TH, xlnT, lnp, "a")

                        # ---- QKV ----
                        # halo k/v columns: one psum tile = 16 blocks x 2
                        ph = tp_ps.tile([P, 8, 4], f32, tag="tp", name="ph")
                        for j in range(16):
                            col = D + j * P
                            wt = qkvw[col // 768]
                            wo = col % 768
                            for i in range(KQ // 2):
                                xc = (2 * i) % 8
                                nc.tensor.matmul(
                                    ph[:, j // 2, (j % 2) * 2:(j % 2) * 2 + 2],
                                    wt[:, 2 * i:2 * i + 2, wo:wo + P],
                                    xlnT[:, xc:xc + 2, 0:2],
                                    start=(i == 0), stop=(i == KQ // 2 - 1),
                                    perf_mode=DR,
                                )
                        pht = ln_pool.tile([P, 32], f32, tag="pht",
                                           name="pht")
                        nc.vector.tensor_mul(pht[:], ph[:, :, :],
                                             cp("khs", 0, 32))
                        for j in range(16):
                            nc.gpsimd.tensor_add(
                                kvT[:, j, 0:2], pht[:, 2 * j:2 * j + 2],
                                cp("khb", 2 * j, 2))

                        def qkv_tile(j):
                            wt = qkvw[j // 6]
                            wo = (j % 6) * P
                            ps = mm_ps.tile([P, T], f32, tag="mm",
                                            name=f"qkv{j}")
                            for i in range(KQ // 2):
                                xc = (2 * i) % 8
                                nc.tensor.matmul(
                                    ps[:], wt[:, 2 * i:2 * i + 2, wo:wo + P],
                                    xlnT[:, xc:xc + 2, 2:TH],
                                    start=(i == 0), stop=(i == KQ // 2 - 1),
                                    perf_mode=DR,
                                )
                            if j < 8:
                                dst = qT[:, j * T:(j + 1) * T]
                            else:
                                dst = kvT[:, j - 8, 2:TH]
                            if j % 2 == 0:
                                nc.vector.tensor_scalar(
                                    dst, ps[:], cp("qkvs", j), cp("qkvb", j),
                                    ALU.mult, ALU.add)
                            else:
                                nc.scalar.activation(dst, ps[:], ACT.Identity,
                                                     bias=cp("qkvb", j),
                                                     scale=cp("qkvs", j))

                        for j in range(16):      # q then k
                            qkv_tile(j)
                        # scores overlap the v-tile matmuls below; e-muls
                        # split DVE / Pool
                        for w in range(3):
                            e = p2.tile([P, 4, T], bf16, tag="e", bufs=2,
                                        name=f"e{w}")
                            e2 = p2.tile([P, 4, T], bf16, tag="e", bufs=2,
                                         name=f"e2{w}")
                            nc.vector.tensor_mul(
                                e[:], qT[:, 0:4 * T],
                                kvT[:, 0:4, 2 - w:2 - w + T])
                            nc.gpsimd.tensor_mul(
                                e2[:], qT[:, 4 * T:8 * T],
                                kvT[:, 4:8, 2 - w:2 - w + T])
                            sc = sc_ps.tile([H, T], f32, tag="sc",
                                            name=f"sc{w}")
                            for ch in range(8):
                                esrc = e if ch < 4 else e2
                                nc.tensor.matmul(
                                    sc[:], hmask[:, ch * H:(ch + 1) * H],
                                    esrc[:, ch % 4, :],
                                    start=(ch == 0), stop=(ch == 7),
                                )
                            nc.scalar.activation(et[:, w, :], sc[:], ACT.Exp)
                        # preload the sqrt act table for LN2 while Act
                        # has slack (identity is in every set)
                        scr = ln_pool.tile([P, 1], f32, tag="scr", name="scr")
                        nc.scalar.activation(scr[:], cp("qkvs", 0), ACT.Sqrt)
                        # ---- softmax ----
                        nc.gpsimd.tensor_mul(et[:, 1, 0:1], et[:, 1, 0:1],
                                             emk[:, 0:1])
                        nc.gpsimd.tensor_mul(et[:, 2, 0:2], et[:, 2, 0:2],
                                             emk[:, 1:3])
                        z0 = p2.tile([H, T], bf16, tag="z0", name="z0")
                        z1 = p2.tile([H, T], bf16, tag="z1", name="z1")
                        rz = p2.tile([H, T], bf16, tag="z0", name="rz")
                        nc.gpsimd.tensor_add(z0[:], et[:, 0, :], et[:, 1, :])
                        nc.gpsimd.tensor_add(z1[:], z0[:], et[:, 2, :])
                        with nc.allow_low_precision(reason="softmax bf16"):
                            nc.vector.reciprocal(rz[:], z1[:])
                        for w in range(3):
                            nc.vector.tensor_mul(et[:, w, :], et[:, w, :],
                                                 rz[:])
                        # probs partition-broadcast: SBUF -> DRAM, then ONE
                        # stride-0 DMA fans head rows out (channels are
                        # head-minor: head = partition // 8 in every chunk)
                        nc.scalar.dma_start(pw_dram[:], et[:, :, :])
                        bcs = p2.tile([P, 3, T], bf16, tag="bcs", name="bcs")
                        src = cap.AP(pw_dram, 0,
                                     [[3 * T, H], [0, 8], [1, 3 * T]])
                        nc.scalar.dma_start(bcs[:], src)
                        for j in range(16, 24):  # v
                            qkv_tile(j)

                        for chp in range(4):  # chunk pairs, fully streamed
                            ch = 2 * chp
                            avs = []
                            for w in range(3):
                                av = p2.tile([P, 2, T], bf16, tag="av",
                                             bufs=4, name=f"av{chp}_{w}")
                                for c in range(2):
                                    nc.vector.tensor_mul(
                                        av[:, c, :], bcs[:, w, :],
                                        kvT[:, 8 + ch + c,
                                            2 - w:2 - w + T],
                                    )
                                avs.append(av)
                            av01 = p2.tile([P, 2, T], bf16, tag="av01",
                                           bufs=2, name=f"av01_{chp}")
                            eng = nc.vector if chp == 3 else nc.gpsimd
                            eng.tensor_add(av01[:], avs[0][:], avs[1][:])
                            eng.tensor_add(attnT[:, ch:ch + 2, :],
                                           av01[:], avs[2][:])

                # ---- proj + residual 1 + LN2 (all channel-major) ----
                with tc.tile_pool(name="p5", bufs=1) as p5:
                    pjps = {}
                    for j in range(8):
                        pool, tag = [(sc_ps, "sc"), (mm_ps, "mm"),
                                     (tp_ps, "tp")][0 if j < 2 else
                                                    (1 if j < 6 else 2)]
                        pjps[j] = pool.tile([P, T], f32, tag=tag,
                                            name=f"pj{j}")
                    for i in range(KP // 2):
                        for j in range(8):
                            nc.tensor.matmul(
                                pjps[j][:], projw[:, 2 * i:2 * i + 2,
                                                  j * P:(j + 1) * P],
                                attnT[:, 2 * i:2 * i + 2, :],
                                start=(i == 0), stop=(i == KP // 2 - 1),
                                perf_mode=DR,
                            )
                    # fused evict + scale + residual: x2T = pj*s + x.
                    # Even groups via DVE STT; odd via Act evict + Pool add.
                    ytmp = p5.tile([P, 4, T], bf16, tag="ytmp", name="ytmp")
                    for j in range(8):
                        if j % 2 == 0:
                            nc.vector.scalar_tensor_tensor(
                                x2T[:, j, :], pjps[j][:], cp("projs", j),
                                xT[:, j, :], ALU.mult, ALU.add)
                        else:
                            nc.scalar.activation(ytmp[:, j // 2, :],
                                                 pjps[j][:], ACT.Identity,
                                                 scale=cp("projs", j))
                            nc.gpsimd.tensor_add(x2T[:, j, :],
                                                 ytmp[:, j // 2, :],
                                                 xT[:, j, :])
                    # ---- LN2 ----
                    ln_chan(x2T[:, :, :], T, x2lnT, p5, "b")
                    scr2 = ln_pool.tile([P, 1], f32, tag="scr", name="scr2")
                    nc.scalar.activation(scr2[:], cp("qkvs", 0), ACT.Gelu)

        # ---- MLP fc1 + gelu, fc2 + residual 2 + store ----
        # fc1 tiles rotate on tp_ps; fc2 keeps 6 psum groups live on
        # mm_ps+sc_ps for the whole phase, its i-step lagging the fc1
        # round that produced those hT chunks by one round so the
        # in-order PE queue never stalls on a gelu eviction.
        with tc.tile_pool(name="w1", bufs=1) as w1_pool:
                mT = w1_pool.tile([P, 8 * T], bf16, tag="mT", name="mT")
                hT = w1_pool.tile([P, NCH_H, T], fp8, tag="hT", name="hT")
                if bias_f:
                    nc.vector.memset(hT[:, 32, :], 1.0)
                    nc.vector.memzero(hT[:, 33, :])

                def f2_mm(ps, j, i):
                    wt = fc2w[j // 4]
                    wo = (j % 4) * P
                    nc.tensor.matmul(
                        ps[:], wt[:, 2 * i:2 * i + 2, wo:wo + P],
                        hT[:, 2 * i:2 * i + 2, :],
                        start=(i == 0), stop=(i == K2 // 2 - 1),
                        perf_mode=DR,
                    )

                # fused evict + scale + residual, then store the chunk.
                # Even groups: DVE STT from psum; odd: Act evict + Pool add.
                mtmp = w1_pool.tile([P, 4, T], bf16, tag="mtmp", name="mtmp")

                def f2_evict(ps, j):
                    if j % 2 == 0:
                        nc.vector.scalar_tensor_tensor(
                            mT[:, j * T:(j + 1) * T], ps[:], cp("fc2s", j),
                            x2T[:, j, :], ALU.mult, ALU.add)
                    else:
                        nc.scalar.activation(mtmp[:, j // 2, :], ps[:],
                                             ACT.Identity,
                                             scale=cp("fc2s", j))
                        nc.gpsimd.tensor_add(mT[:, j * T:(j + 1) * T],
                                             mtmp[:, j // 2, :],
                                             x2T[:, j, :])
                    nc.sync.dma_start(out_d[j * P:(j + 1) * P, :],
                                      mT[:, j * T:(j + 1) * T])

                f2ps = {}
                for j in range(6):
                    pool = sc_ps if j < 2 else mm_ps
                    f2ps[j] = pool.tile([P, T], f32,
                                        tag="sc" if j < 2 else "mm",
                                        name=f"f2{j}")

                for r in range(16):
                    for jj in (2 * r, 2 * r + 1):
                        wt = fc1w[jj // 16]
                        wo = (jj % 16) * P
                        ps = tp_ps.tile([P, T], f32, tag="tp", name=f"f1{jj}")
                        for i in range(K1 // 2):
                            xc = (2 * i) % 8
                            nc.tensor.matmul(
                                ps[:], wt[:, 2 * i:2 * i + 2, wo:wo + P],
                                x2lnT[:, xc:xc + 2, :],
                                start=(i == 0), stop=(i == K1 // 2 - 1),
                                perf_mode=DR,
                            )
                        nc.scalar.activation(hT[:, jj, :], ps[:], ACT.Gelu,
                                             bias=cp("fc1b", jj),
                                             scale=cp("fc1s", jj))
                    if r >= 1:
                        for j in range(6):
                            f2_mm(f2ps[j][:], j, r - 1)
                for j in range(6):
                    for i in range(15, K2 // 2):
                        f2_mm(f2ps[j][:], j, i)
                    f2_evict(f2ps[j][:], j)
                for j in (6, 7):
                    ps = tp_ps.tile([P, T], f32, tag="tp", name=f"f2{j}")
                    for i in range(K2 // 2):
                        f2_mm(ps[:], j, i)
                    f2_evict(ps[:], j)

    if not nc.is_finalized():
        nc.finalize()
    return nc


def _scale_w(w):
    amax = np.abs(w).max(axis=0, keepdims=True)
    s = 2.0 ** np.round(np.log2(2.0 / np.maximum(amax, 1e-30)))
    return w * s, (1.0 / s)[0]


def _prep_w(w, comp):
    """[Din, Dout] fp32 -> ([128, kchunks, Dout] fp8 chunk-major hi(+lo),
    descale vector [Dout])."""
    din, dout = w.shape
    nch = din // P
    ws, descale = _scale_w(np.ascontiguousarray(w.astype(np.float32)))
    hi = ws.astype(F8)
    blocks = [hi]
    if comp:
        lo = (ws - hi.astype(np.float32)).astype(F8)
        blocks.append(lo)
    cols = []
    for b in blocks:
        cols.append(b.reshape(nch, P, dout).transpose(1, 0, 2))
    out = np.concatenate(cols, axis=1)  # [128, kchunks, dout]
    return np.ascontiguousarray(out), descale.astype(np.float32)


def _perm():
    """Head-minor channel permutation: new channel k*128 + h*8 + j holds
    old channel h*64 + k*8 + j, so head(partition p) = p // 8 in every
    chunk of the transposed layout."""
    p = np.empty(D, np.int64)
    for k in range(8):
        for h in range(H):
            for j in range(8):
                p[k * P + h * 8 + j] = h * HD + k * 8 + j
    return p


def _host_inputs(x, qkv_w, qkv_b, proj_w, proj_b, g1, b1, g2, b2,
                 fc1_w, fc1_b, fc2_w, fc2_b):
    scale = HD ** -0.5
    qkvw_eff = (qkv_w * g1[:, None]).astype(np.float32).copy()
    qkvb_eff = (qkv_b + b1 @ qkv_w).astype(np.float32).copy()
    qkvw_eff[:, 0:D] *= scale
    qkvb_eff[0:D] *= scale
    pm = _perm()
    for s in range(3):
        qkvw_eff[:, s * D:(s + 1) * D] = qkvw_eff[:, s * D + pm]
        qkvb_eff[s * D:(s + 1) * D] = qkvb_eff[s * D + pm]
    proj_w = np.ascontiguousarray(proj_w[pm, :]).astype(np.float32)
    fc2_w = np.asarray(fc2_w, np.float32)
    bias_p = bool(np.any(proj_b))
    bias_f = bool(np.any(fc2_b))
    if bias_p:  # ones-chunk pair: extra moving chunk of 1s picks up b/128
        proj_w = np.vstack([proj_w, np.tile(proj_b[None, :] / P, (P, 1)),
                            np.zeros((P, D), np.float32)])
    if bias_f:
        fc2_w = np.vstack([fc2_w, np.tile(fc2_b[None, :] / P, (P, 1)),
                           np.zeros((P, D), np.float32)])
    fc1w_eff = (fc1_w * g2[:, None]).astype(np.float32)
    fc1b_eff = (fc1_b + b2 @ fc1_w).astype(np.float32)

    qkvw_p, qkvs_v = _prep_w(qkvw_eff, COMP["qkv"])
    projw_p, projs_v = _prep_w(proj_w, COMP["proj"])
    fc1w_p, fc1s_v = _prep_w(fc1w_eff, COMP["fc1"])
    fc2w_p, fc2s_v = _prep_w(fc2_w, COMP["fc2"])

    cpak = np.zeros((P, CPAK_W), np.float32)

    def setc(name, vec, n):
        cpak[:, _C[name]:_C[name] + n] = vec.reshape(n, P).T

    setc("qkvb", qkvb_eff, 24)
    setc("qkvs", qkvs_v, 24)
    setc("projs", projs_v, 8)
    setc("fc1b", fc1b_eff, 32)
    setc("fc1s", fc1s_v, 32)
    setc("fc2s", fc2s_v, 8)
    kv_s = qkvs_v[D:3 * D].reshape(16, P)
    kv_b = qkvb_eff[D:3 * D].reshape(16, P)
    for j in range(16):
        for c in range(2):
            cpak[:, _C["khs"] + 2 * j + c] = kv_s[j]
            cpak[:, _C["khb"] + 2 * j + c] = kv_b[j]

    bpak0 = np.zeros((P, BPAK_W), np.float32)
    hm = np.zeros((P, 8, H), np.float32)
    for c in range(P):
        for ch in range(8):
            hm[c, ch, c // 8] = 1.0
    bpak0[:, _B["hmask"]:_B["hmask"] + 128] = hm.reshape(P, 8 * H)

    common = {
        "projw": np.ascontiguousarray(projw_p.reshape(P, -1)),
        "cpak": cpak,
    }
    for b in range(4):
        common[f"qkvw{b}"] = np.ascontiguousarray(
            qkvw_p[:, :, b * 768:(b + 1) * 768].reshape(P, -1))
    for b in range(2):
        common[f"fc1w{b}"] = np.ascontiguousarray(
            fc1w_p[:, :, b * 2048:(b + 1) * 2048].reshape(P, -1))
    for b in range(2):
        common[f"fc2w{b}"] = np.ascontiguousarray(
            fc2w_p[:, :, b * 512:(b + 1) * 512].reshape(P, -1))

    in_maps = []
    for core in range(NCORE):
        b, q = divmod(core, 4)
        xa = np.zeros((TH, D), np.float32)
        xa[2:] = x[b, q * T:(q + 1) * T, :]
        bpak = bpak0.copy()
        if q > 0:
            xa[0:2] = x[b, q * T - 2:q * T, :]
            bpak[0:H, _B["emk"]:_B["emk"] + 3] = 1.0
        # channel-major with halo prepended: xmT[p, ch, t]
        xmT = np.ascontiguousarray(
            xa.T.reshape(8, P, TH).transpose(1, 0, 2)).astype(BF)
        m = dict(common)
        m["xmT"] = xmT.reshape(P, -1)
        m["bpak"] = bpak.astype(BF)
        in_maps.append(m)
    return in_maps


def kernel(**inputs) -> np.ndarray:
    from concourse.bass_utils import run_bass_kernel_spmd

    key = (bool(np.any(inputs["proj_b"])), bool(np.any(inputs["fc2_b"])))
    if key not in _CACHE:
        _CACHE[key] = _build_program(bias_p=key[0], bias_f=key[1])
    nc = _CACHE[key]
    in_maps = _host_inputs(**inputs)
    res = run_bass_kernel_spmd(nc, in_maps, list(range(NCORE)))
    outs = res.results
    full = np.zeros((2, 2048, D), np.float32)
    for core in range(NCORE):
        b, q = divmod(core, 4)
        full[b, q * T:(q + 1) * T, :] = outs[core]["out"].astype(
            np.float32).T
    return full
